# revision 8
# baseline (speedup 1.0000x reference)
"""Trainium2 Bass kernel for nn_Node_attention_layer (ragged_sequence).

Full-input contract: kernel(**inputs) takes the unsharded inputs and returns
(out [B,S,HID] f32, k_scores [B,S] f32), matching the reference.

Sharding: data-parallel over batch B=16 across 8 NeuronCores (2 samples per
core); Linear weights replicated; emb_table rows gathered host-side (only
K=16 rows per sample are used).

Per-core device program (SPMD, identical program, per-core data):
  proj   = tanh(x @ W_att + b_att)                    [N=64, D=512]
  projW  = proj @ W_hid[D:] + b_hid                   [64, 512]
           (reassociation: dot_x @ Wh2 == scores @ (proj @ Wh2); softmax rows
            sum to 1 so the +b_hid fold is exact)
  per macro tile of 512 s rows:
    lgT = [projT | kembT].T @ encT                    [80, 512] f32r (~tf32)
    PE-transpose back to [128, 80] tiles; mask-add -1e18 rows; softmax
    over 64 / over 16 (pair-batched vector ops); k_scores = 1/sum(exp)
    out = tanh(encT.T @ Wh1 + scoresT.T @ projW)      f32r matmuls

enc input and out output use host-packed per-macro layouts so every DMA
moves 8KB (4KB bf16) contiguous per partition.
"""

import sys

if "/opt/trn_rl_repo" not in sys.path:
    sys.path.insert(0, "/opt/trn_rl_repo")

import numpy as np

B, S, D, N, K, V = 16, 2048, 512, 64, 16, 32000
HID = 512
NEG = -1e18
N_CORES = 8
BPC = B // N_CORES  # samples per core
S_MACRO = 512       # s columns per enc staging tile
S_SUB = 128         # s rows per compute tile
NMT = S // S_MACRO      # 4 macro tiles per sample
NST = S_MACRO // S_SUB  # 4 sub tiles per macro
NK = N + K              # 80

OUT_BF16 = True     # stage the tanh output as bf16 (halves write traffic)

TRACE = False       # test.py sets True to collect exec_time_ns
LAST_RESULT = {}    # test.py reads exec_time_ns etc from here

_CACHE = {}


def _install_ntff_shim():
    """Provide antenv.axon_hooks (missing in this image) so that
    run_bass_kernel_spmd(trace=True) can collect NTFF profiles via the
    axon PJRT .so's C ABI."""
    import types
    import ctypes
    import contextlib

    if "antenv.axon_hooks" in sys.modules:
        return
    mod = types.ModuleType("antenv.axon_hooks")
    state = {"hook": None}

    def set_axon_ntff_profile_hook(h):
        state["hook"] = h

    def get_axon_ntff_profile_hook():
        return state["hook"]

    mod.set_axon_ntff_profile_hook = set_axon_ntff_profile_hook
    mod.get_axon_ntff_profile_hook = get_axon_ntff_profile_hook
    sys.modules["antenv.axon_hooks"] = mod
    try:
        import antenv

        antenv.axon_hooks = mod
    except ImportError:
        pass

    so_path = "/opt/axon/libaxon_pjrt.so"
    try:
        lib = ctypes.CDLL(so_path)
        if not hasattr(lib, "axon_start_nrt_profile"):
            return
    except OSError:
        return
    lib.axon_start_nrt_profile.argtypes = [
        ctypes.POINTER(ctypes.c_int64), ctypes.c_size_t]
    lib.axon_start_nrt_profile.restype = ctypes.c_int64
    lib.axon_stop_nrt_profile.argtypes = [ctypes.c_char_p]
    lib.axon_stop_nrt_profile.restype = ctypes.c_int64

    @contextlib.contextmanager
    def _hook(output_dir, device_ids):
        import jax

        jax.devices()
        if device_ids:
            ids = (ctypes.c_int64 * len(device_ids))(*device_ids)
            rc = lib.axon_start_nrt_profile(ids, len(device_ids))
        else:
            rc = lib.axon_start_nrt_profile(None, 0)
        if rc != 0:
            raise RuntimeError(f"axon_start_nrt_profile rc={rc}")
        try:
            yield
        finally:
            n = lib.axon_stop_nrt_profile(str(output_dir).encode())
            print(f"ntff profile: {n} file(s) written to {output_dir}",
                  file=sys.stderr)

    set_axon_ntff_profile_hook(_hook)


def _build():
    import concourse.bacc as bacc
    import concourse.mybir as mybir
    import concourse.tile as tile
    from concourse.masks import make_identity
    import concourse.bass as bass

    f32 = mybir.dt.float32
    f32r = mybir.dt.float32r
    bf16 = mybir.dt.bfloat16
    out_dt = bf16 if OUT_BF16 else f32
    AF = mybir.ActivationFunctionType
    AX = mybir.AxisListType
    OP = mybir.AluOpType

    nc = bacc.Bacc("TRN2", target_bir_lowering=False, debug=False,
                   num_devices=N_CORES)

    # enc host-packed: [b, mt, p, c, s_loc] so each macro load is one DMA
    # with 8KB contiguous per partition
    encN_d = nc.dram_tensor("encN", [BPC, NMT, 128, 4, S_MACRO], f32,
                            kind="ExternalInput").ap()
    xT_d = nc.dram_tensor("xT", [BPC, 2 * D, N], f32, kind="ExternalInput").ap()
    kembT_d = nc.dram_tensor("kembT", [BPC, D, K], f32, kind="ExternalInput").ap()
    madd_d = nc.dram_tensor("madd", [BPC, S], f32, kind="ExternalInput").ap()
    Wa_d = nc.dram_tensor("Wa", [2 * D, D], f32, kind="ExternalInput").ap()
    Wh_d = nc.dram_tensor("Wh", [2 * D, HID], f32, kind="ExternalInput").ap()
    batt_d = nc.dram_tensor("batt", [1, D], f32, kind="ExternalInput").ap()
    bhid_d = nc.dram_tensor("bhid", [1, HID], f32, kind="ExternalInput").ap()
    # out host-packed: [b, mt, p, st, h]; host unscrambles
    out_d = nc.dram_tensor("out", [BPC, NMT, 128, NST, HID], out_dt,
                           kind="ExternalOutput").ap()
    ks_d = nc.dram_tensor("ks", [BPC, S], f32, kind="ExternalOutput").ap()

    xT_v = xT_d.rearrange("b (c p) n -> b p c n", p=128)         # [2,128,8,64]
    kembT_v = kembT_d.rearrange("b (c p) k -> b p c k", p=128)   # [2,128,4,16]
    madd_v = madd_d.rearrange("b (j p) -> b p j", p=128)         # [2,128,16]
    Wa_v = Wa_d.rearrange("(c p) e -> p c e", p=128)             # [128,8,512]
    Wh_v = Wh_d.rearrange("(c p) h -> p c h", p=128)             # [128,8,512]
    ks_v = ks_d.rearrange("b (j p) -> b j p", p=128)             # [2,16,128]

    with tile.TileContext(nc) as tc:
        with tc.tile_pool(name="consts", bufs=1) as consts, \
             tc.tile_pool(name="wstage", bufs=1) as wstage, \
             tc.tile_pool(name="samp", bufs=2) as samp, \
             tc.tile_pool(name="encp", bufs=3) as encp, \
             tc.tile_pool(name="work", bufs=3) as work, \
             tc.tile_pool(name="outp", bufs=2) as outp, \
             tc.tile_pool(name="pslogT", bufs=2, space="PSUM") as pslogT, \
             tc.tile_pool(name="pslg", bufs=2, space="PSUM") as pslg, \
             tc.tile_pool(name="psout", bufs=2, space="PSUM") as psout, \
             tc.tile_pool(name="pst", bufs=2, space="PSUM") as pst:

            ident = consts.tile([128, 128], f32)
            make_identity(nc, ident)
            ident_r = consts.tile([128, 128], f32r)
            nc.vector.tensor_copy(out=ident_r, in_=ident)
            ones_f = consts.tile([1, N], f32)
            nc.vector.memset(ones_f, 1.0)
            ones_r = consts.tile([1, N], f32r)
            nc.vector.tensor_copy(out=ones_r, in_=ones_f)
            batt_st = wstage.tile([1, D], f32)
            nc.sync.dma_start(out=batt_st, in_=batt_d)
            batt_r = consts.tile([1, D], f32r)
            nc.vector.tensor_copy(out=batt_r, in_=batt_st)
            # b_hid broadcast to 64 partitions (stride-0 partition dim)
            bhid_bc = consts.tile([N, HID], f32)
            bhid_bcast_ap = bass.AP(
                tensor=bhid_d.tensor, offset=bhid_d.offset,
                ap=[[0, N], bhid_d.ap[1]],
            )
            nc.sync.dma_start(out=bhid_bc, in_=bhid_bcast_ap)

            Wa_st = wstage.tile([128, 8, 512], f32)
            nc.sync.dma_start(out=Wa_st, in_=Wa_v)
            Wa_r = consts.tile([128, 8, 512], f32r)
            nc.vector.tensor_copy(out=Wa_r, in_=Wa_st)
            Wh_st = wstage.tile([128, 8, 512], f32)
            nc.sync.dma_start(out=Wh_st, in_=Wh_v)
            Wh_r = consts.tile([128, 8, 512], f32r)
            nc.vector.tensor_copy(out=Wh_r, in_=Wh_st)

            for b in range(BPC):
                xT_st = samp.tile([128, 8, N], f32)
                nc.sync.dma_start(out=xT_st, in_=xT_v[b])
                xT_r = samp.tile([128, 8, N], f32r)
                nc.vector.tensor_copy(out=xT_r, in_=xT_st)
                kemb_st = samp.tile([128, 4, K], f32)
                nc.sync.dma_start(out=kemb_st, in_=kembT_v[b])
                madd_t = samp.tile([128, 16], f32)
                nc.sync.dma_start(out=madd_t, in_=madd_v[b])

                # proj = tanh(x @ Wa + b_att)  [64, 512]
                proj_ps = pst.tile([N, D], f32, tag="tp")
                for c in range(8):
                    nc.tensor.matmul(proj_ps, xT_r[:, c, :], Wa_r[:, c, :],
                                     start=(c == 0), stop=False)
                nc.tensor.matmul(proj_ps, ones_r, batt_r, start=False,
                                 stop=True)
                proj_f = samp.tile([N, D], f32)
                nc.scalar.activation(out=proj_f, in_=proj_ps, func=AF.Tanh)

                # pkT[:, c, 0:64] = projT chunk c;  pkT[:, c, 64:80] = kembT
                pkT = samp.tile([128, 4, NK], f32r)
                for c in range(4):
                    pt_ps = pst.tile([128, N], f32, tag="tp")
                    nc.tensor.transpose(
                        pt_ps, proj_f[:, c * 128:(c + 1) * 128],
                        ident[0:N, 0:N])
                    nc.vector.tensor_copy(out=pkT[:, c, 0:N], in_=pt_ps)
                nc.vector.tensor_copy(out=pkT[:, :, N:NK], in_=kemb_st)

                # projW = proj @ Wh2 + b_hid  [64, 512]
                pw_ps = pst.tile([N, HID], f32, tag="tp")
                for c in range(4):
                    nc.tensor.matmul(pw_ps, pkT[:, c, 0:N], Wh_r[:, 4 + c, :],
                                     start=(c == 0), stop=(c == 3))
                # projW duplicated to both partition halves so the scores
                # matmul can use lhsT slices at base partition 0 or 64
                projW_r = samp.tile([2 * N, HID], f32r)
                nc.vector.tensor_add(projW_r[0:N, :], pw_ps, bhid_bc)
                nc.vector.tensor_add(projW_r[N:2 * N, :], pw_ps, bhid_bc)

                kst_sb = samp.tile([128, 16], f32)

                for mt in range(NMT):
                    enc_st = encp.tile([128, 4, S_MACRO], f32)
                    nc.sync.dma_start(out=enc_st, in_=encN_d[b, mt])
                    enc_r = encp.tile([128, 4, S_MACRO], f32r)
                    # split the rounding cast across two engines
                    nc.vector.tensor_copy(out=enc_r[:, 0:2, :],
                                          in_=enc_st[:, 0:2, :])
                    nc.gpsimd.tensor_copy(out=enc_r[:, 2:4, :],
                                          in_=enc_st[:, 2:4, :])

                    # (1) transposed: lgT = [projT|kembT].T @ enc  [80, 512]
                    lgT_ps = pslogT.tile([NK, S_MACRO], f32)
                    for c in range(4):
                        nc.tensor.matmul(lgT_ps, pkT[:, c, :], enc_r[:, c, :],
                                         start=(c == 0), stop=(c == 3))
                    lgT_sb = work.tile([NK, S_MACRO], f32r)
                    nc.vector.tensor_copy(out=lgT_sb, in_=lgT_ps)

                    o4_sb = outp.tile([128, NST, HID], out_dt)

                    for pr in range(NST // 2):
                        j0 = mt * NST + 2 * pr
                        # transpose two subtiles into one [128, 2, 80] psum
                        lg_ps = pslg.tile([128, 2, NK], f32r)
                        for i in range(2):
                            c0 = (2 * pr + i) * S_SUB
                            nc.tensor.transpose(
                                lg_ps[:, i, :], lgT_sb[:, c0:c0 + S_SUB],
                                ident_r[0:NK, 0:NK])

                        # mask (+-1e18 rows) then softmax pieces, pair-batched
                        lg_sb = work.tile([128, 2, NK], f32)
                        nc.vector.tensor_add(
                            lg_sb, lg_ps,
                            madd_t[:, j0:j0 + 2].broadcast_to([128, 2, NK]))
                        nmax = work.tile([128, 2], f32)
                        nc.vector.tensor_reduce(
                            out=nmax, in_=lg_sb[:, :, 0:N], axis=AX.X,
                            op=OP.max, negate=True)
                        kmax = work.tile([128, 2], f32)
                        nc.vector.tensor_reduce(
                            out=kmax, in_=lg_sb[:, :, N:NK], axis=AX.X,
                            op=OP.max, negate=True)
                        e_in = work.tile([128, 2, NK], f32)
                        nc.vector.tensor_add(
                            e_in[:, :, 0:N], lg_sb[:, :, 0:N],
                            nmax.broadcast_to([128, 2, N]))
                        nc.vector.tensor_add(
                            e_in[:, :, N:NK], lg_sb[:, :, N:NK],
                            kmax.broadcast_to([128, 2, K]))
                        e_out = work.tile([128, 2, NK], f32)
                        nc.scalar.activation(out=e_out, in_=e_in, func=AF.Exp)
                        nsum = work.tile([128, 2], f32)
                        nc.vector.tensor_reduce(
                            out=nsum, in_=e_out[:, :, 0:N], axis=AX.X,
                            op=OP.add)
                        ksum = work.tile([128, 2], f32)
                        nc.vector.tensor_reduce(
                            out=ksum, in_=e_out[:, :, N:NK], axis=AX.X,
                            op=OP.add)
                        rn = work.tile([128, 2], f32)
                        nc.vector.reciprocal(out=rn, in_=nsum)
                        nc.vector.reciprocal(out=kst_sb[:, j0:j0 + 2],
                                             in_=ksum)
                        ps_sc = work.tile([128, 2, N], f32r)
                        nc.vector.tensor_mul(
                            ps_sc, e_out[:, :, 0:N],
                            rn.broadcast_to([128, 2, N]))

                        # one [128,128] transpose covers both subtiles:
                        # rows 0:64 = scoresT of subtile 2pr, 64:128 = 2pr+1
                        scT_ps = pst.tile([128, 128], f32r, tag="tp")
                        nc.tensor.transpose(
                            scT_ps, ps_sc.rearrange("p a n -> p (a n)"),
                            ident_r)
                        scT_sb = work.tile([128, 128], f32r)
                        nc.vector.tensor_copy(out=scT_sb, in_=scT_ps)

                        for i in range(2):
                            st = 2 * pr + i
                            sl = slice(st * S_SUB, (st + 1) * S_SUB)
                            o_ps = psout.tile([128, HID], f32)
                            for c in range(4):
                                nc.tensor.matmul(o_ps, enc_r[:, c, sl],
                                                 Wh_r[:, c, :],
                                                 start=(c == 0), stop=False)
                            nc.tensor.matmul(
                                o_ps, scT_sb[i * N:(i + 1) * N, :],
                                projW_r[i * N:(i + 1) * N, :],
                                start=False, stop=True)
                            nc.scalar.activation(out=o4_sb[:, st, :],
                                                 in_=o_ps, func=AF.Tanh)

                    nc.sync.dma_start(out=out_d[b, mt], in_=o4_sb)

                kT_ps = pst.tile([16, 128], f32, tag="tp")
                nc.tensor.transpose(kT_ps, kst_sb, ident)
                kT_sb = samp.tile([16, 128], f32)
                nc.vector.tensor_copy(out=kT_sb, in_=kT_ps)
                nc.sync.dma_start(out=ks_v[b], in_=kT_sb)

    nc.compile()
    return nc


def _get_nc():
    if "nc" not in _CACHE:
        _CACHE["nc"] = _build()
    return _CACHE["nc"]


def kernel(enc_outputs, x, key_concepts, mask_enc, W_att, b_att, W_hid, b_hid,
           emb_table):
    _install_ntff_shim()
    from concourse.bass_utils import run_bass_kernel_spmd

    enc_outputs = np.asarray(enc_outputs, dtype=np.float32)
    x = np.asarray(x, dtype=np.float32)
    key_concepts = np.asarray(key_concepts)
    mask_enc = np.asarray(mask_enc)
    W_att = np.asarray(W_att, dtype=np.float32)
    b_att = np.asarray(b_att, dtype=np.float32)
    W_hid = np.asarray(W_hid, dtype=np.float32)
    b_hid = np.asarray(b_hid, dtype=np.float32)
    emb_table = np.asarray(emb_table, dtype=np.float32)

    scale = np.float32(D ** -0.5)
    # host prep: packed enc layout [b, mt, p, c, s_loc], gather, mask encoding
    # encN[b, mt, p, c, s] = enc[b, mt*512 + s, c*128 + p]
    encN = np.ascontiguousarray(
        enc_outputs.reshape(B, NMT, S_MACRO, 4, 128).transpose(0, 1, 4, 3, 2))
    xT = np.ascontiguousarray(x.transpose(0, 2, 1))               # [B,2D,N]
    k_emb = emb_table[key_concepts.astype(np.int64)] * scale      # [B,K,D]
    kembT = np.ascontiguousarray(k_emb.transpose(0, 2, 1))        # [B,D,K]
    madd = np.where(mask_enc, np.float32(NEG), np.float32(0.0)).astype(np.float32)

    batt = np.ascontiguousarray(b_att.reshape(1, D))
    bhid = np.ascontiguousarray(b_hid.reshape(1, HID))
    Wa = np.ascontiguousarray(W_att)
    Wh = np.ascontiguousarray(W_hid)

    in_maps = []
    for i in range(N_CORES):
        lo, hi = i * BPC, (i + 1) * BPC
        in_maps.append({
            "encN": np.ascontiguousarray(encN[lo:hi]),
            "xT": np.ascontiguousarray(xT[lo:hi]),
            "kembT": np.ascontiguousarray(kembT[lo:hi]),
            "madd": np.ascontiguousarray(madd[lo:hi]),
            "Wa": Wa,
            "Wh": Wh,
            "batt": batt,
            "bhid": bhid,
        })

    nc = _get_nc()
    res = run_bass_kernel_spmd(nc, in_maps, core_ids=list(range(N_CORES)),
                               trace=TRACE)
    LAST_RESULT["exec_time_ns"] = res.exec_time_ns
    LAST_RESULT["mean_exec_time_ns"] = res.mean_exec_time_ns

    # out device layout [BPC, mt, p, st, h] -> [BPC, S, HID]
    outs = []
    for i in range(N_CORES):
        o = np.asarray(res.results[i]["out"])
        if o.dtype != np.float32:
            o = o.astype(np.float32)
        o = o.reshape(BPC, NMT, 128, NST, HID).transpose(0, 1, 3, 2, 4)
        outs.append(o.reshape(BPC, S, HID))
    out = np.concatenate(outs, axis=0)
    ks = np.concatenate([res.results[i]["ks"] for i in range(N_CORES)], axis=0)
    return out, ks


# revision 10
# speedup vs baseline: 1.1086x; 1.1086x over previous
"""Trainium2 Bass kernel for nn_Node_attention_layer (ragged_sequence).

Full-input contract: kernel(**inputs) takes the unsharded inputs and returns
(out [B,S,HID] f32, k_scores [B,S] f32), matching the reference.

Sharding: data-parallel over batch B=16 across 8 NeuronCores (2 samples per
core); Linear weights replicated; emb_table rows gathered host-side (only
K=16 rows per sample are used).

Per-core device program (SPMD, identical program, per-core data):
  proj   = tanh(x @ W_att + b_att)                    [N=64, D=512]
  projW  = proj @ W_hid[D:] + b_hid                   [64, 512]
           (reassociation: dot_x @ Wh2 == scores @ (proj @ Wh2); softmax rows
            sum to 1 so the +b_hid fold is exact)
  per macro tile of 512 s rows:
    lgT = [projT | kembT].T @ encT                    [80, 512] f32r (~tf32)
    PE-transpose back to [128, 80] tiles; mask-add -1e18 rows; softmax
    over 64 / over 16 (pair-batched vector ops); k_scores = 1/sum(exp)
    out = tanh(encT.T @ Wh1 + scoresT.T @ projW)      f32r matmuls

enc input and out output use host-packed per-macro layouts so every DMA
moves 8KB (4KB bf16) contiguous per partition.
"""

import sys

if "/opt/trn_rl_repo" not in sys.path:
    sys.path.insert(0, "/opt/trn_rl_repo")

import numpy as np

B, S, D, N, K, V = 16, 2048, 512, 64, 16, 32000
HID = 512
NEG = -1e18
N_CORES = 8
BPC = B // N_CORES  # samples per core
S_MACRO = 512       # s columns per enc staging tile
S_SUB = 128         # s rows per compute tile
NMT = S // S_MACRO      # 4 macro tiles per sample
NST = S_MACRO // S_SUB  # 4 sub tiles per macro
NK = N + K              # 80

OUT_BF16 = True     # stage the tanh output as bf16 (halves write traffic)

TRACE = False       # test.py sets True to collect exec_time_ns
LAST_RESULT = {}    # test.py reads exec_time_ns etc from here

_CACHE = {}


def _install_ntff_shim():
    """Provide antenv.axon_hooks (missing in this image) so that
    run_bass_kernel_spmd(trace=True) can collect NTFF profiles via the
    axon PJRT .so's C ABI."""
    import types
    import ctypes
    import contextlib

    if "antenv.axon_hooks" in sys.modules:
        return
    mod = types.ModuleType("antenv.axon_hooks")
    state = {"hook": None}

    def set_axon_ntff_profile_hook(h):
        state["hook"] = h

    def get_axon_ntff_profile_hook():
        return state["hook"]

    mod.set_axon_ntff_profile_hook = set_axon_ntff_profile_hook
    mod.get_axon_ntff_profile_hook = get_axon_ntff_profile_hook
    sys.modules["antenv.axon_hooks"] = mod
    try:
        import antenv

        antenv.axon_hooks = mod
    except ImportError:
        pass

    so_path = "/opt/axon/libaxon_pjrt.so"
    try:
        lib = ctypes.CDLL(so_path)
        if not hasattr(lib, "axon_start_nrt_profile"):
            return
    except OSError:
        return
    lib.axon_start_nrt_profile.argtypes = [
        ctypes.POINTER(ctypes.c_int64), ctypes.c_size_t]
    lib.axon_start_nrt_profile.restype = ctypes.c_int64
    lib.axon_stop_nrt_profile.argtypes = [ctypes.c_char_p]
    lib.axon_stop_nrt_profile.restype = ctypes.c_int64

    @contextlib.contextmanager
    def _hook(output_dir, device_ids):
        import jax

        jax.devices()
        if device_ids:
            ids = (ctypes.c_int64 * len(device_ids))(*device_ids)
            rc = lib.axon_start_nrt_profile(ids, len(device_ids))
        else:
            rc = lib.axon_start_nrt_profile(None, 0)
        if rc != 0:
            raise RuntimeError(f"axon_start_nrt_profile rc={rc}")
        try:
            yield
        finally:
            n = lib.axon_stop_nrt_profile(str(output_dir).encode())
            print(f"ntff profile: {n} file(s) written to {output_dir}",
                  file=sys.stderr)

    set_axon_ntff_profile_hook(_hook)


def _build():
    import concourse.bacc as bacc
    import concourse.mybir as mybir
    import concourse.tile as tile
    from concourse.masks import make_identity
    import concourse.bass as bass

    f32 = mybir.dt.float32
    f32r = mybir.dt.float32r
    bf16 = mybir.dt.bfloat16
    out_dt = bf16 if OUT_BF16 else f32
    AF = mybir.ActivationFunctionType
    AX = mybir.AxisListType
    OP = mybir.AluOpType

    nc = bacc.Bacc("TRN2", target_bir_lowering=False, debug=False,
                   num_devices=N_CORES)

    # enc host-packed: [b, mt, p, c, s_loc] so each macro load is one DMA
    # with 8KB contiguous per partition
    encN_d = nc.dram_tensor("encN", [BPC, NMT, 128, 4, S_MACRO], f32,
                            kind="ExternalInput").ap()
    xT_d = nc.dram_tensor("xT", [BPC, 2 * D, N], f32, kind="ExternalInput").ap()
    kembT_d = nc.dram_tensor("kembT", [BPC, D, K], f32, kind="ExternalInput").ap()
    madd_d = nc.dram_tensor("madd", [BPC, S], f32, kind="ExternalInput").ap()
    Wa_d = nc.dram_tensor("Wa", [2 * D, D], f32, kind="ExternalInput").ap()
    Wh_d = nc.dram_tensor("Wh", [2 * D, HID], f32, kind="ExternalInput").ap()
    batt_d = nc.dram_tensor("batt", [1, D], f32, kind="ExternalInput").ap()
    bhid_d = nc.dram_tensor("bhid", [1, HID], f32, kind="ExternalInput").ap()
    # out host-packed: [b, mt, p, st, h]; host unscrambles
    out_d = nc.dram_tensor("out", [BPC, NMT, 128, NST, HID], out_dt,
                           kind="ExternalOutput").ap()
    ks_d = nc.dram_tensor("ks", [BPC, S], f32, kind="ExternalOutput").ap()

    xT_v = xT_d.rearrange("b (c p) n -> b p c n", p=128)         # [2,128,8,64]
    kembT_v = kembT_d.rearrange("b (c p) k -> b p c k", p=128)   # [2,128,4,16]
    madd_v = madd_d.rearrange("b (j p) -> b p j", p=128)         # [2,128,16]
    Wa_v = Wa_d.rearrange("(c p) e -> p c e", p=128)             # [128,8,512]
    Wh_v = Wh_d.rearrange("(c p) h -> p c h", p=128)             # [128,8,512]
    ks_v = ks_d.rearrange("b (j p) -> b j p", p=128)             # [2,16,128]

    with tile.TileContext(nc) as tc:
        with tc.tile_pool(name="consts", bufs=1) as consts, \
             tc.tile_pool(name="wstage", bufs=1) as wstage, \
             tc.tile_pool(name="samp", bufs=2) as samp, \
             tc.tile_pool(name="encp", bufs=3) as encp, \
             tc.tile_pool(name="work", bufs=3) as work, \
             tc.tile_pool(name="outp", bufs=2) as outp, \
             tc.tile_pool(name="pslogT", bufs=1, space="PSUM") as pslogT, \
             tc.tile_pool(name="pslg", bufs=2, space="PSUM") as pslg, \
             tc.tile_pool(name="psout", bufs=3, space="PSUM") as psout, \
             tc.tile_pool(name="pst", bufs=2, space="PSUM") as pst:

            ident = consts.tile([128, 128], f32)
            make_identity(nc, ident)
            ident_r = consts.tile([128, 128], f32r)
            nc.vector.tensor_copy(out=ident_r, in_=ident)
            ones_f = consts.tile([1, N], f32)
            nc.vector.memset(ones_f, 1.0)
            ones_r = consts.tile([1, N], f32r)
            nc.vector.tensor_copy(out=ones_r, in_=ones_f)
            batt_st = wstage.tile([1, D], f32)
            nc.sync.dma_start(out=batt_st, in_=batt_d)
            batt_r = consts.tile([1, D], f32r)
            nc.vector.tensor_copy(out=batt_r, in_=batt_st)
            # b_hid broadcast to 64 partitions (stride-0 partition dim)
            bhid_bc = consts.tile([N, HID], f32)
            bhid_bcast_ap = bass.AP(
                tensor=bhid_d.tensor, offset=bhid_d.offset,
                ap=[[0, N], bhid_d.ap[1]],
            )
            nc.sync.dma_start(out=bhid_bc, in_=bhid_bcast_ap)

            Wa_st = wstage.tile([128, 8, 512], f32)
            nc.sync.dma_start(out=Wa_st, in_=Wa_v)
            Wa_r = consts.tile([128, 8, 512], f32r)
            nc.vector.tensor_copy(out=Wa_r, in_=Wa_st)
            Wh_st = wstage.tile([128, 8, 512], f32)
            nc.sync.dma_start(out=Wh_st, in_=Wh_v)
            Wh_r = consts.tile([128, 8, 512], f32r)
            nc.vector.tensor_copy(out=Wh_r, in_=Wh_st)

            for b in range(BPC):
                xT_st = samp.tile([128, 8, N], f32)
                nc.sync.dma_start(out=xT_st, in_=xT_v[b])
                xT_r = samp.tile([128, 8, N], f32r)
                nc.vector.tensor_copy(out=xT_r, in_=xT_st)
                kemb_st = samp.tile([128, 4, K], f32)
                nc.sync.dma_start(out=kemb_st, in_=kembT_v[b])
                madd_t = samp.tile([128, 16], f32)
                nc.sync.dma_start(out=madd_t, in_=madd_v[b])

                # proj = tanh(x @ Wa + b_att)  [64, 512]
                proj_ps = pst.tile([N, D], f32, tag="tp")
                for c in range(8):
                    nc.tensor.matmul(proj_ps, xT_r[:, c, :], Wa_r[:, c, :],
                                     start=(c == 0), stop=False)
                nc.tensor.matmul(proj_ps, ones_r, batt_r, start=False,
                                 stop=True)
                proj_f = samp.tile([N, D], f32)
                nc.scalar.activation(out=proj_f, in_=proj_ps, func=AF.Tanh)

                # pkT[:, c, 0:64] = projT chunk c;  pkT[:, c, 64:80] = kembT
                pkT = samp.tile([128, 4, NK], f32r)
                for c in range(4):
                    pt_ps = pst.tile([128, N], f32, tag="tp")
                    nc.tensor.transpose(
                        pt_ps, proj_f[:, c * 128:(c + 1) * 128],
                        ident[0:N, 0:N])
                    nc.vector.tensor_copy(out=pkT[:, c, 0:N], in_=pt_ps)
                nc.vector.tensor_copy(out=pkT[:, :, N:NK], in_=kemb_st)

                # projW = proj @ Wh2 + b_hid  [64, 512]
                pw_ps = pst.tile([N, HID], f32, tag="tp")
                for c in range(4):
                    nc.tensor.matmul(pw_ps, pkT[:, c, 0:N], Wh_r[:, 4 + c, :],
                                     start=(c == 0), stop=(c == 3))
                # projW duplicated to both partition halves so the scores
                # matmul can use lhsT slices at base partition 0 or 64
                projW_r = samp.tile([2 * N, HID], f32r)
                nc.vector.tensor_add(projW_r[0:N, :], pw_ps, bhid_bc)
                nc.vector.tensor_add(projW_r[N:2 * N, :], pw_ps, bhid_bc)

                kst_sb = samp.tile([128, 16], f32)

                for mt in range(NMT):
                    enc_st = encp.tile([128, 4, S_MACRO], f32)
                    # two half DMAs so the rounding casts start earlier
                    nc.sync.dma_start(out=enc_st[:, 0:2, :],
                                      in_=encN_d[b, mt, :, 0:2, :])
                    nc.sync.dma_start(out=enc_st[:, 2:4, :],
                                      in_=encN_d[b, mt, :, 2:4, :])
                    enc_r = encp.tile([128, 4, S_MACRO], f32r)
                    # split the rounding cast across two engines
                    nc.vector.tensor_copy(out=enc_r[:, 0:2, :],
                                          in_=enc_st[:, 0:2, :])
                    nc.vector.tensor_copy(out=enc_r[:, 2, :],
                                          in_=enc_st[:, 2, :])
                    nc.gpsimd.tensor_copy(out=enc_r[:, 3, :],
                                          in_=enc_st[:, 3, :])

                    # (1) transposed: lgT = [projT|kembT].T @ enc  [80, 512]
                    lgT_ps = pslogT.tile([NK, S_MACRO], f32)
                    for c in range(4):
                        nc.tensor.matmul(lgT_ps, pkT[:, c, :], enc_r[:, c, :],
                                         start=(c == 0), stop=(c == 3))
                    lgT_sb = work.tile([NK, S_MACRO], f32r)
                    nc.vector.tensor_copy(out=lgT_sb, in_=lgT_ps)

                    o4_sb = outp.tile([128, NST, HID], out_dt)

                    for pr in range(NST // 2):
                        j0 = mt * NST + 2 * pr
                        # transpose two subtiles into one [128, 2, 80] psum
                        lg_ps = pslg.tile([128, 2, NK], f32r)
                        for i in range(2):
                            c0 = (2 * pr + i) * S_SUB
                            nc.tensor.transpose(
                                lg_ps[:, i, :], lgT_sb[:, c0:c0 + S_SUB],
                                ident_r[0:NK, 0:NK])

                        # mask (+-1e18 rows) then softmax pieces, pair-batched
                        lg_sb = work.tile([128, 2, NK], f32)
                        nc.vector.tensor_add(
                            lg_sb, lg_ps,
                            madd_t[:, j0:j0 + 2].broadcast_to([128, 2, NK]))
                        nmax = work.tile([128, 2], f32)
                        nc.vector.tensor_reduce(
                            out=nmax, in_=lg_sb[:, :, 0:N], axis=AX.X,
                            op=OP.max, negate=True)
                        kmax = work.tile([128, 2], f32)
                        nc.vector.tensor_reduce(
                            out=kmax, in_=lg_sb[:, :, N:NK], axis=AX.X,
                            op=OP.max, negate=True)
                        e_in = work.tile([128, 2, NK], f32)
                        nc.vector.tensor_add(
                            e_in[:, :, 0:N], lg_sb[:, :, 0:N],
                            nmax.broadcast_to([128, 2, N]))
                        nc.vector.tensor_add(
                            e_in[:, :, N:NK], lg_sb[:, :, N:NK],
                            kmax.broadcast_to([128, 2, K]))
                        e_out = work.tile([128, 2, NK], f32)
                        nc.scalar.activation(out=e_out, in_=e_in, func=AF.Exp)
                        nsum = work.tile([128, 2], f32)
                        nc.vector.tensor_reduce(
                            out=nsum, in_=e_out[:, :, 0:N], axis=AX.X,
                            op=OP.add)
                        ksum = work.tile([128, 2], f32)
                        nc.vector.tensor_reduce(
                            out=ksum, in_=e_out[:, :, N:NK], axis=AX.X,
                            op=OP.add)
                        rn = work.tile([128, 2], f32)
                        nc.vector.reciprocal(out=rn, in_=nsum)
                        nc.vector.reciprocal(out=kst_sb[:, j0:j0 + 2],
                                             in_=ksum)
                        ps_sc = work.tile([128, 2, N], f32r)
                        nc.vector.tensor_mul(
                            ps_sc, e_out[:, :, 0:N],
                            rn.broadcast_to([128, 2, N]))

                        # one [128,128] transpose covers both subtiles:
                        # rows 0:64 = scoresT of subtile 2pr, 64:128 = 2pr+1
                        scT_ps = pst.tile([128, 128], f32r, tag="tp")
                        nc.tensor.transpose(
                            scT_ps, ps_sc.rearrange("p a n -> p (a n)"),
                            ident_r)
                        scT_sb = work.tile([128, 128], f32r)
                        nc.vector.tensor_copy(out=scT_sb, in_=scT_ps)

                        for i in range(2):
                            st = 2 * pr + i
                            sl = slice(st * S_SUB, (st + 1) * S_SUB)
                            o_ps = psout.tile([128, HID], f32)
                            for c in range(4):
                                nc.tensor.matmul(o_ps, enc_r[:, c, sl],
                                                 Wh_r[:, c, :],
                                                 start=(c == 0), stop=False)
                            nc.tensor.matmul(
                                o_ps, scT_sb[i * N:(i + 1) * N, :],
                                projW_r[i * N:(i + 1) * N, :],
                                start=False, stop=True)
                            nc.scalar.activation(out=o4_sb[:, st, :],
                                                 in_=o_ps, func=AF.Tanh)

                    nc.sync.dma_start(out=out_d[b, mt], in_=o4_sb)

                kT_ps = pst.tile([16, 128], f32, tag="tp")
                nc.tensor.transpose(kT_ps, kst_sb, ident)
                kT_sb = samp.tile([16, 128], f32)
                nc.vector.tensor_copy(out=kT_sb, in_=kT_ps)
                nc.sync.dma_start(out=ks_v[b], in_=kT_sb)

    nc.compile()
    return nc


def _get_nc():
    if "nc" not in _CACHE:
        _CACHE["nc"] = _build()
    return _CACHE["nc"]


def kernel(enc_outputs, x, key_concepts, mask_enc, W_att, b_att, W_hid, b_hid,
           emb_table):
    _install_ntff_shim()
    from concourse.bass_utils import run_bass_kernel_spmd

    enc_outputs = np.asarray(enc_outputs, dtype=np.float32)
    x = np.asarray(x, dtype=np.float32)
    key_concepts = np.asarray(key_concepts)
    mask_enc = np.asarray(mask_enc)
    W_att = np.asarray(W_att, dtype=np.float32)
    b_att = np.asarray(b_att, dtype=np.float32)
    W_hid = np.asarray(W_hid, dtype=np.float32)
    b_hid = np.asarray(b_hid, dtype=np.float32)
    emb_table = np.asarray(emb_table, dtype=np.float32)

    scale = np.float32(D ** -0.5)
    # host prep: packed enc layout [b, mt, p, c, s_loc], gather, mask encoding
    # encN[b, mt, p, c, s] = enc[b, mt*512 + s, c*128 + p]
    encN = np.ascontiguousarray(
        enc_outputs.reshape(B, NMT, S_MACRO, 4, 128).transpose(0, 1, 4, 3, 2))
    xT = np.ascontiguousarray(x.transpose(0, 2, 1))               # [B,2D,N]
    k_emb = emb_table[key_concepts.astype(np.int64)] * scale      # [B,K,D]
    kembT = np.ascontiguousarray(k_emb.transpose(0, 2, 1))        # [B,D,K]
    madd = np.where(mask_enc, np.float32(NEG), np.float32(0.0)).astype(np.float32)

    batt = np.ascontiguousarray(b_att.reshape(1, D))
    bhid = np.ascontiguousarray(b_hid.reshape(1, HID))
    Wa = np.ascontiguousarray(W_att)
    Wh = np.ascontiguousarray(W_hid)

    in_maps = []
    for i in range(N_CORES):
        lo, hi = i * BPC, (i + 1) * BPC
        in_maps.append({
            "encN": np.ascontiguousarray(encN[lo:hi]),
            "xT": np.ascontiguousarray(xT[lo:hi]),
            "kembT": np.ascontiguousarray(kembT[lo:hi]),
            "madd": np.ascontiguousarray(madd[lo:hi]),
            "Wa": Wa,
            "Wh": Wh,
            "batt": batt,
            "bhid": bhid,
        })

    nc = _get_nc()
    res = run_bass_kernel_spmd(nc, in_maps, core_ids=list(range(N_CORES)),
                               trace=TRACE)
    LAST_RESULT["exec_time_ns"] = res.exec_time_ns
    LAST_RESULT["mean_exec_time_ns"] = res.mean_exec_time_ns

    # out device layout [BPC, mt, p, st, h] -> [BPC, S, HID]
    outs = []
    for i in range(N_CORES):
        o = np.asarray(res.results[i]["out"])
        if o.dtype != np.float32:
            o = o.astype(np.float32)
        o = o.reshape(BPC, NMT, 128, NST, HID).transpose(0, 1, 3, 2, 4)
        outs.append(o.reshape(BPC, S, HID))
    out = np.concatenate(outs, axis=0)
    ks = np.concatenate([res.results[i]["ks"] for i in range(N_CORES)], axis=0)
    return out, ks


# revision 15
# speedup vs baseline: 1.1236x; 1.0135x over previous
"""Trainium2 Bass kernel for nn_Node_attention_layer (ragged_sequence).

Full-input contract: kernel(**inputs) takes the unsharded inputs and returns
(out [B,S,HID] f32, k_scores [B,S] f32), matching the reference.

Sharding: data-parallel over batch B=16 across 8 NeuronCores (2 samples per
core); Linear weights replicated; emb_table rows gathered host-side (only
K=16 rows per sample are used).

Per-core device program (SPMD, identical program, per-core data):
  proj   = tanh(x @ W_att + b_att)                    [N=64, D=512]
  projW  = proj @ W_hid[D:] + b_hid                   [64, 512]
           (reassociation: dot_x @ Wh2 == scores @ (proj @ Wh2); softmax rows
            sum to 1 so the +b_hid fold is exact)
  per macro tile of 512 s rows:
    lgT = [projT | kembT].T @ encT                    [80, 512] f32r (~tf32)
    PE-transpose back to [128, 80] tiles; mask-add -1e18 rows; softmax
    over 64 / over 16 (pair-batched vector ops); k_scores = 1/sum(exp)
    out = tanh(encT.T @ Wh1 + scoresT.T @ projW)      f32r matmuls

enc input and out output use host-packed per-macro layouts so every DMA
moves 8KB (4KB bf16) contiguous per partition.
"""

import sys

if "/opt/trn_rl_repo" not in sys.path:
    sys.path.insert(0, "/opt/trn_rl_repo")

import numpy as np

B, S, D, N, K, V = 16, 2048, 512, 64, 16, 32000
HID = 512
NEG = -1e18
N_CORES = 8
BPC = B // N_CORES  # samples per core
S_MACRO = 512       # s columns per enc staging tile
S_SUB = 128         # s rows per compute tile
NMT = S // S_MACRO      # 4 macro tiles per sample
NST = S_MACRO // S_SUB  # 4 sub tiles per macro
NK = N + K              # 80

OUT_BF16 = True     # stage the tanh output as bf16 (halves write traffic)

TRACE = False       # test.py sets True to collect exec_time_ns
LAST_RESULT = {}    # test.py reads exec_time_ns etc from here

_CACHE = {}


def _install_ntff_shim():
    """Provide antenv.axon_hooks (missing in this image) so that
    run_bass_kernel_spmd(trace=True) can collect NTFF profiles via the
    axon PJRT .so's C ABI."""
    import types
    import ctypes
    import contextlib

    if "antenv.axon_hooks" in sys.modules:
        return
    mod = types.ModuleType("antenv.axon_hooks")
    state = {"hook": None}

    def set_axon_ntff_profile_hook(h):
        state["hook"] = h

    def get_axon_ntff_profile_hook():
        return state["hook"]

    mod.set_axon_ntff_profile_hook = set_axon_ntff_profile_hook
    mod.get_axon_ntff_profile_hook = get_axon_ntff_profile_hook
    sys.modules["antenv.axon_hooks"] = mod
    try:
        import antenv

        antenv.axon_hooks = mod
    except ImportError:
        pass

    so_path = "/opt/axon/libaxon_pjrt.so"
    try:
        lib = ctypes.CDLL(so_path)
        if not hasattr(lib, "axon_start_nrt_profile"):
            return
    except OSError:
        return
    lib.axon_start_nrt_profile.argtypes = [
        ctypes.POINTER(ctypes.c_int64), ctypes.c_size_t]
    lib.axon_start_nrt_profile.restype = ctypes.c_int64
    lib.axon_stop_nrt_profile.argtypes = [ctypes.c_char_p]
    lib.axon_stop_nrt_profile.restype = ctypes.c_int64

    @contextlib.contextmanager
    def _hook(output_dir, device_ids):
        import jax

        jax.devices()
        if device_ids:
            ids = (ctypes.c_int64 * len(device_ids))(*device_ids)
            rc = lib.axon_start_nrt_profile(ids, len(device_ids))
        else:
            rc = lib.axon_start_nrt_profile(None, 0)
        if rc != 0:
            raise RuntimeError(f"axon_start_nrt_profile rc={rc}")
        try:
            yield
        finally:
            n = lib.axon_stop_nrt_profile(str(output_dir).encode())
            print(f"ntff profile: {n} file(s) written to {output_dir}",
                  file=sys.stderr)

    set_axon_ntff_profile_hook(_hook)


def _build():
    import concourse.bacc as bacc
    import concourse.mybir as mybir
    import concourse.tile as tile
    from concourse.masks import make_identity
    import concourse.bass as bass

    f32 = mybir.dt.float32
    f32r = mybir.dt.float32r
    bf16 = mybir.dt.bfloat16
    out_dt = bf16 if OUT_BF16 else f32
    AF = mybir.ActivationFunctionType
    AX = mybir.AxisListType
    OP = mybir.AluOpType

    nc = bacc.Bacc("TRN2", target_bir_lowering=False, debug=False,
                   num_devices=N_CORES)

    # enc host-packed: [b, mt, p, c, s_loc] so each macro load is one DMA
    # with 8KB contiguous per partition
    encN_d = nc.dram_tensor("encN", [BPC, NMT, 128, 4, S_MACRO], f32,
                            kind="ExternalInput").ap()
    xT_d = nc.dram_tensor("xT", [BPC, 2 * D, N], f32, kind="ExternalInput").ap()
    kembT_d = nc.dram_tensor("kembT", [BPC, D, K], f32, kind="ExternalInput").ap()
    madd_d = nc.dram_tensor("madd", [BPC, S], f32, kind="ExternalInput").ap()
    Wa_d = nc.dram_tensor("Wa", [2 * D, D], f32, kind="ExternalInput").ap()
    Wh_d = nc.dram_tensor("Wh", [2 * D, HID], f32, kind="ExternalInput").ap()
    batt_d = nc.dram_tensor("batt", [1, D], f32, kind="ExternalInput").ap()
    bhid_d = nc.dram_tensor("bhid", [1, HID], f32, kind="ExternalInput").ap()
    # out host-packed: [b, mt, p, st, h]; host unscrambles
    out_d = nc.dram_tensor("out", [BPC, NMT, 128, NST, HID], out_dt,
                           kind="ExternalOutput").ap()
    ks_d = nc.dram_tensor("ks", [BPC, S], f32, kind="ExternalOutput").ap()

    xT_v = xT_d.rearrange("b (c p) n -> b p c n", p=128)         # [2,128,8,64]
    kembT_v = kembT_d.rearrange("b (c p) k -> b p c k", p=128)   # [2,128,4,16]
    madd_v = madd_d.rearrange("b (j p) -> b p j", p=128)         # [2,128,16]
    Wa_v = Wa_d.rearrange("(c p) e -> p c e", p=128)             # [128,8,512]
    Wh_v = Wh_d.rearrange("(c p) h -> p c h", p=128)             # [128,8,512]
    ks_v = ks_d.rearrange("b (j p) -> b j p", p=128)             # [2,16,128]

    with tile.TileContext(nc) as tc:
        with tc.tile_pool(name="consts", bufs=1) as consts, \
             tc.tile_pool(name="wstage", bufs=1) as wstage, \
             tc.tile_pool(name="samp", bufs=2) as samp, \
             tc.tile_pool(name="encp", bufs=3) as encp, \
             tc.tile_pool(name="work", bufs=3) as work, \
             tc.tile_pool(name="outp", bufs=2) as outp, \
             tc.tile_pool(name="pslogT", bufs=1, space="PSUM") as pslogT, \
             tc.tile_pool(name="pslg", bufs=2, space="PSUM") as pslg, \
             tc.tile_pool(name="psout", bufs=3, space="PSUM") as psout, \
             tc.tile_pool(name="pst", bufs=2, space="PSUM") as pst:

            ident = consts.tile([128, 128], f32)
            make_identity(nc, ident)
            ident_r = consts.tile([128, 128], f32r)
            nc.vector.tensor_copy(out=ident_r, in_=ident)
            ones_f = consts.tile([1, NK], f32)
            nc.vector.memset(ones_f, 1.0)
            ones_r = consts.tile([1, NK], f32r)
            nc.vector.tensor_copy(out=ones_r, in_=ones_f)

            # HAM warm-up: ~4us of dense matmuls while the first DMAs
            # stream, so the PE clock is at 2.4GHz when real work arrives
            warm_ps = pst.tile([128, 128], f32, tag="tp")
            for i in range(20):
                nc.tensor.matmul(warm_ps, ident, ident,
                                 start=(i == 0), stop=(i == 19))
            batt_st = wstage.tile([1, D], f32)
            nc.sync.dma_start(out=batt_st, in_=batt_d)
            batt_r = consts.tile([1, D], f32r)
            nc.vector.tensor_copy(out=batt_r, in_=batt_st)
            # b_hid broadcast to 64 partitions (stride-0 partition dim)
            bhid_bc = consts.tile([N, HID], f32)
            bhid_bcast_ap = bass.AP(
                tensor=bhid_d.tensor, offset=bhid_d.offset,
                ap=[[0, N], bhid_d.ap[1]],
            )
            nc.sync.dma_start(out=bhid_bc, in_=bhid_bcast_ap)

            Wa_st = wstage.tile([128, 8, 512], f32)
            nc.sync.dma_start(out=Wa_st, in_=Wa_v)
            Wa_r = consts.tile([128, 8, 512], f32r)
            nc.vector.tensor_copy(out=Wa_r, in_=Wa_st)
            Wh_st = wstage.tile([128, 8, 512], f32)
            nc.sync.dma_start(out=Wh_st, in_=Wh_v)
            Wh_r = consts.tile([128, 8, 512], f32r)
            nc.vector.tensor_copy(out=Wh_r, in_=Wh_st)

            pkT_b = []
            projW_b = []
            maddT_b = []
            kst_b = []
            for b in range(BPC):
                xT_st = samp.tile([128, 8, N], f32)
                nc.sync.dma_start(out=xT_st, in_=xT_v[b])
                xT_r = samp.tile([128, 8, N], f32r)
                nc.vector.tensor_copy(out=xT_r, in_=xT_st)
                kemb_st = samp.tile([128, 4, K], f32)
                nc.sync.dma_start(out=kemb_st, in_=kembT_v[b])
                # madd as a free-dim row vector for the PE mask-bias matmul
                maddT_st = samp.tile([1, S], f32)
                nc.sync.dma_start(out=maddT_st, in_=madd_d[b:b + 1, :])
                maddT_b.append(maddT_st)

                # proj = tanh(x @ Wa + b_att)  [64, 512]
                proj_ps = pst.tile([N, D], f32, tag="tp")
                for c in range(8):
                    nc.tensor.matmul(proj_ps, xT_r[:, c, :], Wa_r[:, c, :],
                                     start=(c == 0), stop=False)
                nc.tensor.matmul(proj_ps, ones_r[:, 0:N], batt_r, start=False,
                                 stop=True)
                proj_f = samp.tile([N, D], f32)
                nc.scalar.activation(out=proj_f, in_=proj_ps, func=AF.Tanh)

                # pkT[:, c, 0:64] = projT chunk c;  pkT[:, c, 64:80] = kembT
                pkT = samp.tile([128, 4, NK], f32r)
                for c in range(4):
                    pt_ps = pst.tile([128, N], f32, tag="tp")
                    nc.tensor.transpose(
                        pt_ps, proj_f[:, c * 128:(c + 1) * 128],
                        ident[0:N, 0:N])
                    nc.vector.tensor_copy(out=pkT[:, c, 0:N], in_=pt_ps)
                nc.vector.tensor_copy(out=pkT[:, :, N:NK], in_=kemb_st)
                pkT_b.append(pkT)

                # projW = proj @ Wh2 + b_hid  [64, 512]
                pw_ps = pst.tile([N, HID], f32, tag="tp")
                for c in range(4):
                    nc.tensor.matmul(pw_ps, pkT[:, c, 0:N], Wh_r[:, 4 + c, :],
                                     start=(c == 0), stop=(c == 3))
                # projW duplicated to both partition halves so the scores
                # matmul can use lhsT slices at base partition 0 or 64
                projW_r = samp.tile([2 * N, HID], f32r)
                nc.vector.tensor_add(projW_r[0:N, :], pw_ps, bhid_bc)
                nc.vector.tensor_add(projW_r[N:2 * N, :], pw_ps, bhid_bc)
                projW_b.append(projW_r)

                kst_sb = samp.tile([128, 16], f32, tag="kst")
                kst_b.append(kst_sb)

            for b in range(BPC):
                pkT = pkT_b[b]
                projW_r = projW_b[b]
                maddT = maddT_b[b]
                kst_sb = kst_b[b]
                for mt in range(NMT):
                    enc_st = encp.tile([128, 4, S_MACRO], f32)
                    # two half DMAs so the rounding casts start earlier
                    nc.sync.dma_start(out=enc_st[:, 0:2, :],
                                      in_=encN_d[b, mt, :, 0:2, :])
                    nc.sync.dma_start(out=enc_st[:, 2:4, :],
                                      in_=encN_d[b, mt, :, 2:4, :])
                    enc_r = encp.tile([128, 4, S_MACRO], f32r)
                    # split the rounding cast across two engines
                    nc.vector.tensor_copy(out=enc_r[:, 0:2, :],
                                          in_=enc_st[:, 0:2, :])
                    nc.vector.tensor_copy(out=enc_r[:, 2, :],
                                          in_=enc_st[:, 2, :])
                    nc.gpsimd.tensor_copy(out=enc_r[:, 3, :],
                                          in_=enc_st[:, 3, :])

                    # (1) transposed: lgT = [projT|kembT].T @ enc  [80, 512]
                    lgT_ps = pslogT.tile([NK, S_MACRO], f32)
                    for c in range(4):
                        nc.tensor.matmul(lgT_ps, pkT[:, c, :], enc_r[:, c, :],
                                         start=(c == 0), stop=(c == 3))
                    lgT_sb = work.tile([NK, S_MACRO], f32)
                    nc.vector.tensor_copy(out=lgT_sb, in_=lgT_ps)

                    o4_sb = outp.tile([128, NST, HID], out_dt)

                    for pr in range(NST // 2):
                        j0 = mt * NST + 2 * pr
                        # mask bias (outer product madd x ones, K=1 matmul)
                        # + logits transpose accumulate into [128, 2, 80]
                        lg_ps = pslg.tile([128, 2, NK], f32)
                        for i in range(2):
                            s0 = (j0 + i) * S_SUB
                            c0 = (2 * pr + i) * S_SUB
                            nc.tensor.matmul(
                                lg_ps[:, i, :], maddT[:, s0:s0 + S_SUB],
                                ones_f, start=True, stop=False)
                            nc.tensor.matmul(
                                lg_ps[:, i, :], lgT_sb[:, c0:c0 + S_SUB],
                                ident[0:NK, 0:NK], is_transpose=True,
                                start=False, stop=True)

                        # softmax pieces, pair-batched, reading psum direct
                        nmax = work.tile([128, 2], f32)
                        nc.vector.tensor_reduce(
                            out=nmax, in_=lg_ps[:, :, 0:N], axis=AX.X,
                            op=OP.max, negate=True)
                        kmax = work.tile([128, 2], f32)
                        nc.vector.tensor_reduce(
                            out=kmax, in_=lg_ps[:, :, N:NK], axis=AX.X,
                            op=OP.max, negate=True)
                        e_in = work.tile([128, 2, NK], f32)
                        nc.vector.tensor_add(
                            e_in[:, :, 0:N], lg_ps[:, :, 0:N],
                            nmax.broadcast_to([128, 2, N]))
                        nc.vector.tensor_add(
                            e_in[:, :, N:NK], lg_ps[:, :, N:NK],
                            kmax.broadcast_to([128, 2, K]))
                        e_out = work.tile([128, 2, NK], f32)
                        nc.scalar.activation(out=e_out, in_=e_in, func=AF.Exp)
                        nsum = work.tile([128, 2], f32)
                        nc.vector.tensor_reduce(
                            out=nsum, in_=e_out[:, :, 0:N], axis=AX.X,
                            op=OP.add)
                        ksum = work.tile([128, 2], f32)
                        nc.vector.tensor_reduce(
                            out=ksum, in_=e_out[:, :, N:NK], axis=AX.X,
                            op=OP.add)
                        rn = work.tile([128, 2], f32)
                        nc.vector.reciprocal(out=rn, in_=nsum)
                        nc.vector.reciprocal(out=kst_sb[:, j0:j0 + 2],
                                             in_=ksum)
                        ps_sc = work.tile([128, 2, N], f32r)
                        nc.vector.tensor_mul(
                            ps_sc, e_out[:, :, 0:N],
                            rn.broadcast_to([128, 2, N]))

                        # one [128,128] transpose covers both subtiles:
                        # rows 0:64 = scoresT of subtile 2pr, 64:128 = 2pr+1
                        scT_ps = pst.tile([128, 128], f32r, tag="tp")
                        nc.tensor.transpose(
                            scT_ps, ps_sc.rearrange("p a n -> p (a n)"),
                            ident_r)
                        scT_sb = work.tile([128, 128], f32r)
                        nc.vector.tensor_copy(out=scT_sb, in_=scT_ps)

                        for i in range(2):
                            st = 2 * pr + i
                            sl = slice(st * S_SUB, (st + 1) * S_SUB)
                            o_ps = psout.tile([128, HID], f32)
                            for c in range(4):
                                nc.tensor.matmul(o_ps, enc_r[:, c, sl],
                                                 Wh_r[:, c, :],
                                                 start=(c == 0), stop=False)
                            nc.tensor.matmul(
                                o_ps, scT_sb[i * N:(i + 1) * N, :],
                                projW_r[i * N:(i + 1) * N, :],
                                start=False, stop=True)
                            nc.scalar.activation(out=o4_sb[:, st, :],
                                                 in_=o_ps, func=AF.Tanh)

                    nc.sync.dma_start(out=out_d[b, mt], in_=o4_sb)

                kT_ps = pst.tile([16, 128], f32, tag="tp")
                nc.tensor.transpose(kT_ps, kst_sb, ident)
                kT_sb = samp.tile([16, 128], f32)
                nc.vector.tensor_copy(out=kT_sb, in_=kT_ps)
                nc.sync.dma_start(out=ks_v[b], in_=kT_sb)

    nc.compile()
    return nc


def _get_nc():
    if "nc" not in _CACHE:
        _CACHE["nc"] = _build()
    return _CACHE["nc"]


def kernel(enc_outputs, x, key_concepts, mask_enc, W_att, b_att, W_hid, b_hid,
           emb_table):
    _install_ntff_shim()
    from concourse.bass_utils import run_bass_kernel_spmd

    enc_outputs = np.asarray(enc_outputs, dtype=np.float32)
    x = np.asarray(x, dtype=np.float32)
    key_concepts = np.asarray(key_concepts)
    mask_enc = np.asarray(mask_enc)
    W_att = np.asarray(W_att, dtype=np.float32)
    b_att = np.asarray(b_att, dtype=np.float32)
    W_hid = np.asarray(W_hid, dtype=np.float32)
    b_hid = np.asarray(b_hid, dtype=np.float32)
    emb_table = np.asarray(emb_table, dtype=np.float32)

    scale = np.float32(D ** -0.5)
    # host prep: packed enc layout [b, mt, p, c, s_loc], gather, mask encoding
    # encN[b, mt, p, c, s] = enc[b, mt*512 + s, c*128 + p]
    encN = np.ascontiguousarray(
        enc_outputs.reshape(B, NMT, S_MACRO, 4, 128).transpose(0, 1, 4, 3, 2))
    xT = np.ascontiguousarray(x.transpose(0, 2, 1))               # [B,2D,N]
    k_emb = emb_table[key_concepts.astype(np.int64)] * scale      # [B,K,D]
    kembT = np.ascontiguousarray(k_emb.transpose(0, 2, 1))        # [B,D,K]
    madd = np.where(mask_enc, np.float32(NEG), np.float32(0.0)).astype(np.float32)

    batt = np.ascontiguousarray(b_att.reshape(1, D))
    bhid = np.ascontiguousarray(b_hid.reshape(1, HID))
    Wa = np.ascontiguousarray(W_att)
    Wh = np.ascontiguousarray(W_hid)

    in_maps = []
    for i in range(N_CORES):
        lo, hi = i * BPC, (i + 1) * BPC
        in_maps.append({
            "encN": np.ascontiguousarray(encN[lo:hi]),
            "xT": np.ascontiguousarray(xT[lo:hi]),
            "kembT": np.ascontiguousarray(kembT[lo:hi]),
            "madd": np.ascontiguousarray(madd[lo:hi]),
            "Wa": Wa,
            "Wh": Wh,
            "batt": batt,
            "bhid": bhid,
        })

    nc = _get_nc()
    res = run_bass_kernel_spmd(nc, in_maps, core_ids=list(range(N_CORES)),
                               trace=TRACE)
    LAST_RESULT["exec_time_ns"] = res.exec_time_ns
    LAST_RESULT["mean_exec_time_ns"] = res.mean_exec_time_ns

    # out device layout [BPC, mt, p, st, h] -> [BPC, S, HID]
    outs = []
    for i in range(N_CORES):
        o = np.asarray(res.results[i]["out"])
        if o.dtype != np.float32:
            o = o.astype(np.float32)
        o = o.reshape(BPC, NMT, 128, NST, HID).transpose(0, 1, 3, 2, 4)
        outs.append(o.reshape(BPC, S, HID))
    out = np.concatenate(outs, axis=0)
    ks = np.concatenate([res.results[i]["ks"] for i in range(N_CORES)], axis=0)
    return out, ks


# revision 17
# speedup vs baseline: 1.1492x; 1.0228x over previous
"""Trainium2 Bass kernel for nn_Node_attention_layer (ragged_sequence).

Full-input contract: kernel(**inputs) takes the unsharded inputs and returns
(out [B,S,HID] f32, k_scores [B,S] f32), matching the reference.

Sharding: data-parallel over batch B=16 across 8 NeuronCores (2 samples per
core); Linear weights replicated; emb_table rows gathered host-side (only
K=16 rows per sample are used).

Per-core device program (SPMD, identical program, per-core data):
  proj   = tanh(x @ W_att + b_att)                    [N=64, D=512]
  projW  = proj @ W_hid[D:] + b_hid                   [64, 512]
           (reassociation: dot_x @ Wh2 == scores @ (proj @ Wh2); softmax rows
            sum to 1 so the +b_hid fold is exact)
  per macro tile of 512 s rows:
    lgT = [projT | kembT].T @ encT                    [80, 512] f32r (~tf32)
    PE-transpose back to [128, 80] tiles; mask-add -1e18 rows; softmax
    over 64 / over 16 (pair-batched vector ops); k_scores = 1/sum(exp)
    out = tanh(encT.T @ Wh1 + scoresT.T @ projW)      f32r matmuls

enc input and out output use host-packed per-macro layouts so every DMA
moves 8KB (4KB bf16) contiguous per partition.
"""

import sys

if "/opt/trn_rl_repo" not in sys.path:
    sys.path.insert(0, "/opt/trn_rl_repo")

import numpy as np

B, S, D, N, K, V = 16, 2048, 512, 64, 16, 32000
HID = 512
NEG = -1e18
N_CORES = 8
BPC = B // N_CORES  # samples per core
S_MACRO = 512       # s columns per enc staging tile
S_SUB = 128         # s rows per compute tile
NMT = S // S_MACRO      # 4 macro tiles per sample
NST = S_MACRO // S_SUB  # 4 sub tiles per macro
NK = N + K              # 80

OUT_BF16 = True     # stage the tanh output as bf16 (halves write traffic)

TRACE = False       # test.py sets True to collect exec_time_ns
LAST_RESULT = {}    # test.py reads exec_time_ns etc from here

_CACHE = {}


def _install_ntff_shim():
    """Provide antenv.axon_hooks (missing in this image) so that
    run_bass_kernel_spmd(trace=True) can collect NTFF profiles via the
    axon PJRT .so's C ABI."""
    import types
    import ctypes
    import contextlib

    if "antenv.axon_hooks" in sys.modules:
        return
    mod = types.ModuleType("antenv.axon_hooks")
    state = {"hook": None}

    def set_axon_ntff_profile_hook(h):
        state["hook"] = h

    def get_axon_ntff_profile_hook():
        return state["hook"]

    mod.set_axon_ntff_profile_hook = set_axon_ntff_profile_hook
    mod.get_axon_ntff_profile_hook = get_axon_ntff_profile_hook
    sys.modules["antenv.axon_hooks"] = mod
    try:
        import antenv

        antenv.axon_hooks = mod
    except ImportError:
        pass

    so_path = "/opt/axon/libaxon_pjrt.so"
    try:
        lib = ctypes.CDLL(so_path)
        if not hasattr(lib, "axon_start_nrt_profile"):
            return
    except OSError:
        return
    lib.axon_start_nrt_profile.argtypes = [
        ctypes.POINTER(ctypes.c_int64), ctypes.c_size_t]
    lib.axon_start_nrt_profile.restype = ctypes.c_int64
    lib.axon_stop_nrt_profile.argtypes = [ctypes.c_char_p]
    lib.axon_stop_nrt_profile.restype = ctypes.c_int64

    @contextlib.contextmanager
    def _hook(output_dir, device_ids):
        import jax

        jax.devices()
        if device_ids:
            ids = (ctypes.c_int64 * len(device_ids))(*device_ids)
            rc = lib.axon_start_nrt_profile(ids, len(device_ids))
        else:
            rc = lib.axon_start_nrt_profile(None, 0)
        if rc != 0:
            raise RuntimeError(f"axon_start_nrt_profile rc={rc}")
        try:
            yield
        finally:
            n = lib.axon_stop_nrt_profile(str(output_dir).encode())
            print(f"ntff profile: {n} file(s) written to {output_dir}",
                  file=sys.stderr)

    set_axon_ntff_profile_hook(_hook)


def _build():
    import concourse.bacc as bacc
    import concourse.mybir as mybir
    import concourse.tile as tile
    from concourse.masks import make_identity
    import concourse.bass as bass

    f32 = mybir.dt.float32
    f32r = mybir.dt.float32r
    bf16 = mybir.dt.bfloat16
    out_dt = bf16 if OUT_BF16 else f32
    AF = mybir.ActivationFunctionType
    AX = mybir.AxisListType
    OP = mybir.AluOpType

    nc = bacc.Bacc("TRN2", target_bir_lowering=False, debug=False,
                   num_devices=N_CORES)

    # enc host-packed: [b, mt, p, c, s_loc] so each macro load is one DMA
    # with 8KB contiguous per partition
    encN_d = nc.dram_tensor("encN", [BPC, NMT, 128, 4, S_MACRO], f32,
                            kind="ExternalInput").ap()
    xT_d = nc.dram_tensor("xT", [BPC, 2 * D, N], f32, kind="ExternalInput").ap()
    kembT_d = nc.dram_tensor("kembT", [BPC, D, K], f32, kind="ExternalInput").ap()
    madd_d = nc.dram_tensor("madd", [BPC, S], f32, kind="ExternalInput").ap()
    Wa_d = nc.dram_tensor("Wa", [2 * D, D], f32, kind="ExternalInput").ap()
    Wh_d = nc.dram_tensor("Wh", [2 * D, HID], f32, kind="ExternalInput").ap()
    batt_d = nc.dram_tensor("batt", [1, D], f32, kind="ExternalInput").ap()
    bhid_d = nc.dram_tensor("bhid", [1, HID], f32, kind="ExternalInput").ap()
    # out host-packed: [b, mt, p, st, h]; host unscrambles
    out_d = nc.dram_tensor("out", [BPC, NMT, 128, NST, HID], out_dt,
                           kind="ExternalOutput").ap()
    ks_d = nc.dram_tensor("ks", [BPC, S], f32, kind="ExternalOutput").ap()

    xT_v = xT_d.rearrange("b (c p) n -> b p c n", p=128)         # [2,128,8,64]
    kembT_v = kembT_d.rearrange("b (c p) k -> b p c k", p=128)   # [2,128,4,16]
    madd_v = madd_d.rearrange("b (j p) -> b p j", p=128)         # [2,128,16]
    Wa_v = Wa_d.rearrange("(c p) e -> p c e", p=128)             # [128,8,512]
    Wh_v = Wh_d.rearrange("(c p) h -> p c h", p=128)             # [128,8,512]
    ks_v = ks_d.rearrange("b (j p) -> b j p", p=128)             # [2,16,128]

    with tile.TileContext(nc) as tc:
        with tc.tile_pool(name="consts", bufs=1) as consts, \
             tc.tile_pool(name="wstage", bufs=1) as wstage, \
             tc.tile_pool(name="samp", bufs=2) as samp, \
             tc.tile_pool(name="encp", bufs=3) as encp, \
             tc.tile_pool(name="work", bufs=3) as work, \
             tc.tile_pool(name="outp", bufs=2) as outp, \
             tc.tile_pool(name="pslogT", bufs=1, space="PSUM") as pslogT, \
             tc.tile_pool(name="pslg", bufs=2, space="PSUM") as pslg, \
             tc.tile_pool(name="psout", bufs=3, space="PSUM") as psout, \
             tc.tile_pool(name="pst", bufs=2, space="PSUM") as pst:

            ident = consts.tile([128, 128], f32)
            make_identity(nc, ident)
            ident_r = consts.tile([128, 128], f32r)
            nc.vector.tensor_copy(out=ident_r, in_=ident)
            ones_f = consts.tile([1, NK], f32)
            nc.vector.memset(ones_f, 1.0)
            ones_r = consts.tile([1, NK], f32r)
            nc.vector.tensor_copy(out=ones_r, in_=ones_f)

            ident_bf = consts.tile([128, 128], bf16)
            nc.vector.tensor_copy(out=ident_bf, in_=ident)
            # HAM warm-up: ~4us of dense matmuls while the first DMAs
            # stream, so the PE clock is at 2.4GHz when real work arrives
            warm_ps = pst.tile([128, 128], f32, tag="tp")
            for i in range(36):
                nc.tensor.matmul(warm_ps, ident_bf, ident_bf,
                                 start=(i == 0), stop=(i == 35))
            batt_st = wstage.tile([1, D], f32)
            nc.sync.dma_start(out=batt_st, in_=batt_d)
            batt_r = consts.tile([1, D], f32r)
            nc.vector.tensor_copy(out=batt_r, in_=batt_st)
            # b_hid broadcast to 64 partitions (stride-0 partition dim)
            bhid_bc = consts.tile([N, HID], f32)
            bhid_bcast_ap = bass.AP(
                tensor=bhid_d.tensor, offset=bhid_d.offset,
                ap=[[0, N], bhid_d.ap[1]],
            )
            nc.sync.dma_start(out=bhid_bc, in_=bhid_bcast_ap)

            Wa_st = wstage.tile([128, 8, 512], f32)
            nc.sync.dma_start(out=Wa_st, in_=Wa_v)
            Wa_r = consts.tile([128, 8, 512], f32r)
            nc.vector.tensor_copy(out=Wa_r, in_=Wa_st)
            Wh_st = wstage.tile([128, 8, 512], f32)
            nc.sync.dma_start(out=Wh_st, in_=Wh_v)
            Wh_r = consts.tile([128, 8, 512], f32r)
            nc.vector.tensor_copy(out=Wh_r, in_=Wh_st)

            pkT_b = []
            projW_b = []
            maddT_b = []
            kst_b = []
            for b in range(BPC):
                xT_st = samp.tile([128, 8, N], f32)
                nc.sync.dma_start(out=xT_st, in_=xT_v[b])
                xT_r = samp.tile([128, 8, N], f32r)
                nc.vector.tensor_copy(out=xT_r, in_=xT_st)
                kemb_st = samp.tile([128, 4, K], f32)
                nc.sync.dma_start(out=kemb_st, in_=kembT_v[b])
                # madd as a free-dim row vector for the PE mask-bias matmul
                maddT_st = samp.tile([1, S], f32)
                nc.sync.dma_start(out=maddT_st, in_=madd_d[b:b + 1, :])
                maddT = samp.tile([1, S], f32r)
                nc.vector.tensor_copy(out=maddT, in_=maddT_st)
                maddT_b.append(maddT)

                # proj = tanh(x @ Wa + b_att)  [64, 512]
                proj_ps = pst.tile([N, D], f32, tag="tp")
                for c in range(8):
                    nc.tensor.matmul(proj_ps, xT_r[:, c, :], Wa_r[:, c, :],
                                     start=(c == 0), stop=False)
                nc.tensor.matmul(proj_ps, ones_r[:, 0:N], batt_r, start=False,
                                 stop=True)
                proj_f = samp.tile([N, D], f32)
                nc.scalar.activation(out=proj_f, in_=proj_ps, func=AF.Tanh)

                # pkT[:, c, 0:64] = projT chunk c;  pkT[:, c, 64:80] = kembT
                pkT = samp.tile([128, 4, NK], f32r)
                for c in range(4):
                    pt_ps = pst.tile([128, N], f32, tag="tp")
                    nc.tensor.transpose(
                        pt_ps, proj_f[:, c * 128:(c + 1) * 128],
                        ident[0:N, 0:N])
                    nc.vector.tensor_copy(out=pkT[:, c, 0:N], in_=pt_ps)
                nc.vector.tensor_copy(out=pkT[:, :, N:NK], in_=kemb_st)
                pkT_b.append(pkT)

                # projW = proj @ Wh2 + b_hid  [64, 512]
                pw_ps = pst.tile([N, HID], f32, tag="tp")
                for c in range(4):
                    nc.tensor.matmul(pw_ps, pkT[:, c, 0:N], Wh_r[:, 4 + c, :],
                                     start=(c == 0), stop=(c == 3))
                # projW duplicated to both partition halves so the scores
                # matmul can use lhsT slices at base partition 0 or 64
                projW_r = samp.tile([2 * N, HID], f32r)
                nc.vector.tensor_add(projW_r[0:N, :], pw_ps, bhid_bc)
                nc.vector.tensor_add(projW_r[N:2 * N, :], pw_ps, bhid_bc)
                projW_b.append(projW_r)

                kst_sb = samp.tile([128, 16], f32, tag="kst")
                kst_b.append(kst_sb)

            enc_tiles = {}
            for b in range(BPC):
                for mt in range(NMT):
                    enc_st = encp.tile([128, 4, S_MACRO], f32)
                    # two half DMAs so the rounding casts start earlier
                    nc.sync.dma_start(out=enc_st[:, 0:2, :],
                                      in_=encN_d[b, mt, :, 0:2, :])
                    nc.sync.dma_start(out=enc_st[:, 2:4, :],
                                      in_=encN_d[b, mt, :, 2:4, :])
                    enc_r = encp.tile([128, 4, S_MACRO], f32r)
                    # split the rounding cast across two engines
                    nc.vector.tensor_copy(out=enc_r[:, 0:2, :],
                                          in_=enc_st[:, 0:2, :])
                    nc.vector.tensor_copy(out=enc_r[:, 2, :],
                                          in_=enc_st[:, 2, :])
                    nc.gpsimd.tensor_copy(out=enc_r[:, 3, :],
                                          in_=enc_st[:, 3, :])
                    enc_tiles[(b, mt)] = (enc_st, enc_r)

            for b in range(BPC):
                pkT = pkT_b[b]
                projW_r = projW_b[b]
                maddT = maddT_b[b]
                kst_sb = kst_b[b]
                for mt in range(NMT):
                    enc_st, enc_r = enc_tiles[(b, mt)]

                    # (1) transposed: lgT = [projT|kembT].T @ enc + mask
                    # bias (ones80 outer madd_row rides the accumulation)
                    lgT_ps = pslogT.tile([NK, S_MACRO], f32)
                    nc.tensor.matmul(
                        lgT_ps, ones_r,
                        maddT[:, mt * S_MACRO:(mt + 1) * S_MACRO],
                        start=True, stop=False)
                    for c in range(4):
                        nc.tensor.matmul(lgT_ps, pkT[:, c, :], enc_r[:, c, :],
                                         start=False, stop=(c == 3))
                    lgT_sb = work.tile([NK, S_MACRO], f32)
                    nc.vector.tensor_copy(out=lgT_sb, in_=lgT_ps)

                    o4_sb = outp.tile([128, NST, HID], out_dt)

                    for pr in range(NST // 2):
                        j0 = mt * NST + 2 * pr
                        # transpose two subtiles into one [128, 2, 80] psum
                        lg_ps = pslg.tile([128, 2, NK], f32)
                        for i in range(2):
                            c0 = (2 * pr + i) * S_SUB
                            nc.tensor.matmul(
                                lg_ps[:, i, :], lgT_sb[:, c0:c0 + S_SUB],
                                ident[0:NK, 0:NK], is_transpose=True,
                                start=True, stop=True)

                        # softmax pieces, pair-batched, reading psum direct
                        nmax = work.tile([128, 2], f32)
                        nc.vector.tensor_reduce(
                            out=nmax, in_=lg_ps[:, :, 0:N], axis=AX.X,
                            op=OP.max, negate=True)
                        kmax = work.tile([128, 2], f32)
                        nc.vector.tensor_reduce(
                            out=kmax, in_=lg_ps[:, :, N:NK], axis=AX.X,
                            op=OP.max, negate=True)
                        e_in = work.tile([128, 2, NK], f32)
                        nc.vector.tensor_add(
                            e_in[:, :, 0:N], lg_ps[:, :, 0:N],
                            nmax.broadcast_to([128, 2, N]))
                        nc.vector.tensor_add(
                            e_in[:, :, N:NK], lg_ps[:, :, N:NK],
                            kmax.broadcast_to([128, 2, K]))
                        e_out = work.tile([128, 2, NK], f32)
                        nc.scalar.activation(out=e_out, in_=e_in, func=AF.Exp)
                        nsum = work.tile([128, 2], f32)
                        nc.vector.tensor_reduce(
                            out=nsum, in_=e_out[:, :, 0:N], axis=AX.X,
                            op=OP.add)
                        ksum = work.tile([128, 2], f32)
                        nc.vector.tensor_reduce(
                            out=ksum, in_=e_out[:, :, N:NK], axis=AX.X,
                            op=OP.add)
                        rn = work.tile([128, 2], f32)
                        nc.vector.reciprocal(out=rn, in_=nsum)
                        nc.vector.reciprocal(out=kst_sb[:, j0:j0 + 2],
                                             in_=ksum)
                        ps_sc = work.tile([128, 2, N], f32r)
                        nc.vector.tensor_mul(
                            ps_sc, e_out[:, :, 0:N],
                            rn.broadcast_to([128, 2, N]))

                        # one [128,128] transpose covers both subtiles:
                        # rows 0:64 = scoresT of subtile 2pr, 64:128 = 2pr+1
                        scT_ps = pst.tile([128, 128], f32r, tag="tp")
                        nc.tensor.transpose(
                            scT_ps, ps_sc.rearrange("p a n -> p (a n)"),
                            ident_r)
                        scT_sb = work.tile([128, 128], f32r)
                        nc.vector.tensor_copy(out=scT_sb, in_=scT_ps)

                        for i in range(2):
                            st = 2 * pr + i
                            sl = slice(st * S_SUB, (st + 1) * S_SUB)
                            o_ps = psout.tile([128, HID], f32)
                            for c in range(4):
                                nc.tensor.matmul(o_ps, enc_r[:, c, sl],
                                                 Wh_r[:, c, :],
                                                 start=(c == 0), stop=False)
                            nc.tensor.matmul(
                                o_ps, scT_sb[i * N:(i + 1) * N, :],
                                projW_r[i * N:(i + 1) * N, :],
                                start=False, stop=True)
                            nc.scalar.activation(out=o4_sb[:, st, :],
                                                 in_=o_ps, func=AF.Tanh)

                    nc.sync.dma_start(out=out_d[b, mt], in_=o4_sb)

                kT_ps = pst.tile([16, 128], f32, tag="tp")
                nc.tensor.transpose(kT_ps, kst_sb, ident)
                kT_sb = samp.tile([16, 128], f32)
                nc.vector.tensor_copy(out=kT_sb, in_=kT_ps)
                nc.sync.dma_start(out=ks_v[b], in_=kT_sb)

    nc.compile()
    return nc


def _get_nc():
    if "nc" not in _CACHE:
        _CACHE["nc"] = _build()
    return _CACHE["nc"]


def kernel(enc_outputs, x, key_concepts, mask_enc, W_att, b_att, W_hid, b_hid,
           emb_table):
    _install_ntff_shim()
    from concourse.bass_utils import run_bass_kernel_spmd

    enc_outputs = np.asarray(enc_outputs, dtype=np.float32)
    x = np.asarray(x, dtype=np.float32)
    key_concepts = np.asarray(key_concepts)
    mask_enc = np.asarray(mask_enc)
    W_att = np.asarray(W_att, dtype=np.float32)
    b_att = np.asarray(b_att, dtype=np.float32)
    W_hid = np.asarray(W_hid, dtype=np.float32)
    b_hid = np.asarray(b_hid, dtype=np.float32)
    emb_table = np.asarray(emb_table, dtype=np.float32)

    scale = np.float32(D ** -0.5)
    # host prep: packed enc layout [b, mt, p, c, s_loc], gather, mask encoding
    # encN[b, mt, p, c, s] = enc[b, mt*512 + s, c*128 + p]
    encN = np.ascontiguousarray(
        enc_outputs.reshape(B, NMT, S_MACRO, 4, 128).transpose(0, 1, 4, 3, 2))
    xT = np.ascontiguousarray(x.transpose(0, 2, 1))               # [B,2D,N]
    k_emb = emb_table[key_concepts.astype(np.int64)] * scale      # [B,K,D]
    kembT = np.ascontiguousarray(k_emb.transpose(0, 2, 1))        # [B,D,K]
    madd = np.where(mask_enc, np.float32(NEG), np.float32(0.0)).astype(np.float32)

    batt = np.ascontiguousarray(b_att.reshape(1, D))
    bhid = np.ascontiguousarray(b_hid.reshape(1, HID))
    Wa = np.ascontiguousarray(W_att)
    Wh = np.ascontiguousarray(W_hid)

    in_maps = []
    for i in range(N_CORES):
        lo, hi = i * BPC, (i + 1) * BPC
        in_maps.append({
            "encN": np.ascontiguousarray(encN[lo:hi]),
            "xT": np.ascontiguousarray(xT[lo:hi]),
            "kembT": np.ascontiguousarray(kembT[lo:hi]),
            "madd": np.ascontiguousarray(madd[lo:hi]),
            "Wa": Wa,
            "Wh": Wh,
            "batt": batt,
            "bhid": bhid,
        })

    nc = _get_nc()
    res = run_bass_kernel_spmd(nc, in_maps, core_ids=list(range(N_CORES)),
                               trace=TRACE)
    LAST_RESULT["exec_time_ns"] = res.exec_time_ns
    LAST_RESULT["mean_exec_time_ns"] = res.mean_exec_time_ns

    # out device layout [BPC, mt, p, st, h] -> [BPC, S, HID]
    outs = []
    for i in range(N_CORES):
        o = np.asarray(res.results[i]["out"])
        if o.dtype != np.float32:
            o = o.astype(np.float32)
        o = o.reshape(BPC, NMT, 128, NST, HID).transpose(0, 1, 3, 2, 4)
        outs.append(o.reshape(BPC, S, HID))
    out = np.concatenate(outs, axis=0)
    ks = np.concatenate([res.results[i]["ks"] for i in range(N_CORES)], axis=0)
    return out, ks


# revision 18
# speedup vs baseline: 1.4679x; 1.2774x over previous
"""Trainium2 Bass kernel for nn_Node_attention_layer (ragged_sequence).

Full-input contract: kernel(**inputs) takes the unsharded inputs and returns
(out [B,S,HID] f32, k_scores [B,S] f32), matching the reference.

Sharding: data-parallel over batch B=16 across 8 NeuronCores (2 samples per
core); Linear weights replicated; emb_table rows gathered host-side (only
K=16 rows per sample are used).

Per-core device program (SPMD, identical program, per-core data):
  proj   = tanh(x @ W_att + b_att)                    [N=64, D=512]
  projW  = proj @ W_hid[D:] + b_hid                   [64, 512]
           (reassociation: dot_x @ Wh2 == scores @ (proj @ Wh2); softmax rows
            sum to 1 so the +b_hid fold is exact)
  per macro tile of 512 s rows:
    lgT = [projT | kembT].T @ encT + ones80 (x) madd  [80, 512]
          (the mask bias rides the matmul accumulation; -1e18 absorbs the
           row's logits in f32 psum, reproducing masked_fill exactly)
    PE-transpose back to [128, 80] tiles; softmax over 64 / over 16
    (pair-batched vector ops); k_scores = 1/sum(exp(klog - kmax))
    out = tanh(encT.T @ Wh1 + scoresT.T @ projW)

Matmul operands are bf16 (host-converted; softmax/psum math stays f32);
enc/out use host-packed per-macro layouts so every DMA moves >=4KB
contiguous per partition.
"""

import sys

if "/opt/trn_rl_repo" not in sys.path:
    sys.path.insert(0, "/opt/trn_rl_repo")

import numpy as np

B, S, D, N, K, V = 16, 2048, 512, 64, 16, 32000
HID = 512
NEG = -1e18
N_CORES = 8
BPC = B // N_CORES  # samples per core
S_MACRO = 512       # s columns per enc staging tile
S_SUB = 128         # s rows per compute tile
NMT = S // S_MACRO      # 4 macro tiles per sample
NST = S_MACRO // S_SUB  # 4 sub tiles per macro
NK = N + K              # 80

OUT_BF16 = True     # stage the tanh output as bf16 (halves write traffic)

TRACE = False       # test.py sets True to collect exec_time_ns
LAST_RESULT = {}    # test.py reads exec_time_ns etc from here

_CACHE = {}


def _install_ntff_shim():
    """Provide antenv.axon_hooks (missing in this image) so that
    run_bass_kernel_spmd(trace=True) can collect NTFF profiles via the
    axon PJRT .so's C ABI."""
    import types
    import ctypes
    import contextlib

    if "antenv.axon_hooks" in sys.modules:
        return
    mod = types.ModuleType("antenv.axon_hooks")
    state = {"hook": None}

    def set_axon_ntff_profile_hook(h):
        state["hook"] = h

    def get_axon_ntff_profile_hook():
        return state["hook"]

    mod.set_axon_ntff_profile_hook = set_axon_ntff_profile_hook
    mod.get_axon_ntff_profile_hook = get_axon_ntff_profile_hook
    sys.modules["antenv.axon_hooks"] = mod
    try:
        import antenv

        antenv.axon_hooks = mod
    except ImportError:
        pass

    so_path = "/opt/axon/libaxon_pjrt.so"
    try:
        lib = ctypes.CDLL(so_path)
        if not hasattr(lib, "axon_start_nrt_profile"):
            return
    except OSError:
        return
    lib.axon_start_nrt_profile.argtypes = [
        ctypes.POINTER(ctypes.c_int64), ctypes.c_size_t]
    lib.axon_start_nrt_profile.restype = ctypes.c_int64
    lib.axon_stop_nrt_profile.argtypes = [ctypes.c_char_p]
    lib.axon_stop_nrt_profile.restype = ctypes.c_int64

    @contextlib.contextmanager
    def _hook(output_dir, device_ids):
        import jax

        jax.devices()
        if device_ids:
            ids = (ctypes.c_int64 * len(device_ids))(*device_ids)
            rc = lib.axon_start_nrt_profile(ids, len(device_ids))
        else:
            rc = lib.axon_start_nrt_profile(None, 0)
        if rc != 0:
            raise RuntimeError(f"axon_start_nrt_profile rc={rc}")
        try:
            yield
        finally:
            n = lib.axon_stop_nrt_profile(str(output_dir).encode())
            print(f"ntff profile: {n} file(s) written to {output_dir}",
                  file=sys.stderr)

    set_axon_ntff_profile_hook(_hook)


def _build():
    import concourse.bacc as bacc
    import concourse.mybir as mybir
    import concourse.tile as tile
    from concourse.masks import make_identity
    import concourse.bass as bass

    f32 = mybir.dt.float32
    bf16 = mybir.dt.bfloat16
    out_dt = bf16 if OUT_BF16 else f32
    AF = mybir.ActivationFunctionType
    AX = mybir.AxisListType
    OP = mybir.AluOpType

    nc = bacc.Bacc("TRN2", target_bir_lowering=False, debug=False,
                   num_devices=N_CORES)

    # enc host-packed bf16: [b, mt, p, c, s_loc] so each macro load is one
    # DMA with 4KB contiguous per partition
    encN_d = nc.dram_tensor("encN", [BPC, NMT, 128, 4, S_MACRO], bf16,
                            kind="ExternalInput").ap()
    xT_d = nc.dram_tensor("xT", [BPC, 2 * D, N], bf16, kind="ExternalInput").ap()
    kembT_d = nc.dram_tensor("kembT", [BPC, D, K], bf16,
                             kind="ExternalInput").ap()
    madd_d = nc.dram_tensor("madd", [BPC, S], bf16, kind="ExternalInput").ap()
    Wa_d = nc.dram_tensor("Wa", [2 * D, D], bf16, kind="ExternalInput").ap()
    Wh_d = nc.dram_tensor("Wh", [2 * D, HID], bf16, kind="ExternalInput").ap()
    batt_d = nc.dram_tensor("batt", [1, D], bf16, kind="ExternalInput").ap()
    bhid_d = nc.dram_tensor("bhid", [1, HID], f32, kind="ExternalInput").ap()
    # out host-packed: [b, mt, p, st, h]; host unscrambles
    out_d = nc.dram_tensor("out", [BPC, NMT, 128, NST, HID], out_dt,
                           kind="ExternalOutput").ap()
    ks_d = nc.dram_tensor("ks", [BPC, S], f32, kind="ExternalOutput").ap()

    xT_v = xT_d.rearrange("b (c p) n -> b p c n", p=128)         # [2,128,8,64]
    kembT_v = kembT_d.rearrange("b (c p) k -> b p c k", p=128)   # [2,128,4,16]
    Wa_v = Wa_d.rearrange("(c p) e -> p c e", p=128)             # [128,8,512]
    Wh_v = Wh_d.rearrange("(c p) h -> p c h", p=128)             # [128,8,512]
    ks_v = ks_d.rearrange("b (j p) -> b j p", p=128)             # [2,16,128]

    with tile.TileContext(nc) as tc:
        with tc.tile_pool(name="consts", bufs=1) as consts, \
             tc.tile_pool(name="samp", bufs=2) as samp, \
             tc.tile_pool(name="encp", bufs=4) as encp, \
             tc.tile_pool(name="work", bufs=3) as work, \
             tc.tile_pool(name="outp", bufs=2) as outp, \
             tc.tile_pool(name="pslogT", bufs=1, space="PSUM") as pslogT, \
             tc.tile_pool(name="pslg", bufs=2, space="PSUM") as pslg, \
             tc.tile_pool(name="psout", bufs=3, space="PSUM") as psout, \
             tc.tile_pool(name="pst", bufs=2, space="PSUM") as pst:

            ident = consts.tile([128, 128], f32)
            make_identity(nc, ident)
            ident_bf = consts.tile([128, 128], bf16)
            nc.vector.tensor_copy(out=ident_bf, in_=ident)
            ones_bf = consts.tile([1, NK], bf16)
            nc.vector.memset(ones_bf, 1.0)

            # HAM warm-up: ~4us of dense matmuls while the first DMAs
            # stream, so the PE clock is at 2.4GHz when real work arrives
            warm_ps = pst.tile([128, 128], f32, tag="tp")
            for i in range(36):
                nc.tensor.matmul(warm_ps, ident_bf, ident_bf,
                                 start=(i == 0), stop=(i == 35))

            batt_t = consts.tile([1, D], bf16)
            nc.sync.dma_start(out=batt_t, in_=batt_d)
            # b_hid broadcast to 64 partitions (stride-0 partition dim)
            bhid_bc = consts.tile([N, HID], f32)
            bhid_bcast_ap = bass.AP(
                tensor=bhid_d.tensor, offset=bhid_d.offset,
                ap=[[0, N], bhid_d.ap[1]],
            )
            nc.sync.dma_start(out=bhid_bc, in_=bhid_bcast_ap)

            Wa_t = consts.tile([128, 8, 512], bf16)
            nc.sync.dma_start(out=Wa_t, in_=Wa_v)
            Wh_t = consts.tile([128, 8, 512], bf16)
            nc.sync.dma_start(out=Wh_t, in_=Wh_v)

            pkT_b = []
            projW_b = []
            maddT_b = []
            kst_b = []
            for b in range(BPC):
                xT_t = samp.tile([128, 8, N], bf16)
                nc.sync.dma_start(out=xT_t, in_=xT_v[b])
                kemb_t = samp.tile([128, 4, K], bf16)
                nc.sync.dma_start(out=kemb_t, in_=kembT_v[b])
                # madd as a free-dim row vector for the PE mask-bias matmul
                maddT = samp.tile([1, S], bf16)
                nc.sync.dma_start(out=maddT, in_=madd_d[b:b + 1, :])
                maddT_b.append(maddT)

                # proj = tanh(x @ Wa + b_att)  [64, 512]
                proj_ps = pst.tile([N, D], f32, tag="tp")
                for c in range(8):
                    nc.tensor.matmul(proj_ps, xT_t[:, c, :], Wa_t[:, c, :],
                                     start=(c == 0), stop=False)
                nc.tensor.matmul(proj_ps, ones_bf[:, 0:N], batt_t,
                                 start=False, stop=True)
                proj_f = samp.tile([N, D], f32)
                nc.scalar.activation(out=proj_f, in_=proj_ps, func=AF.Tanh)

                # pkT[:, c, 0:64] = projT chunk c;  pkT[:, c, 64:80] = kembT
                pkT = samp.tile([128, 4, NK], bf16)
                for c in range(4):
                    pt_ps = pst.tile([128, N], f32, tag="tp")
                    nc.tensor.transpose(
                        pt_ps, proj_f[:, c * 128:(c + 1) * 128],
                        ident[0:N, 0:N])
                    nc.vector.tensor_copy(out=pkT[:, c, 0:N], in_=pt_ps)
                nc.vector.tensor_copy(out=pkT[:, :, N:NK], in_=kemb_t)
                pkT_b.append(pkT)

                # projW = proj @ Wh2 + b_hid  [64, 512]
                pw_ps = pst.tile([N, HID], f32, tag="tp")
                for c in range(4):
                    nc.tensor.matmul(pw_ps, pkT[:, c, 0:N], Wh_t[:, 4 + c, :],
                                     start=(c == 0), stop=(c == 3))
                # projW duplicated to both partition halves so the scores
                # matmul can use lhsT slices at base partition 0 or 64
                projW_t = samp.tile([2 * N, HID], bf16)
                nc.vector.tensor_add(projW_t[0:N, :], pw_ps, bhid_bc)
                nc.vector.tensor_add(projW_t[N:2 * N, :], pw_ps, bhid_bc)
                projW_b.append(projW_t)

                kst_sb = samp.tile([128, 16], f32, tag="kst")
                kst_b.append(kst_sb)

            enc_tiles = {}
            for b in range(BPC):
                for mt in range(NMT):
                    enc_t = encp.tile([128, 4, S_MACRO], bf16)
                    nc.sync.dma_start(out=enc_t, in_=encN_d[b, mt])
                    enc_tiles[(b, mt)] = enc_t

            for b in range(BPC):
                pkT = pkT_b[b]
                projW_t = projW_b[b]
                maddT = maddT_b[b]
                kst_sb = kst_b[b]
                for mt in range(NMT):
                    enc_t = enc_tiles[(b, mt)]

                    # (1) transposed: lgT = [projT|kembT].T @ enc + mask
                    # bias (ones80 outer madd_row rides the accumulation)
                    lgT_ps = pslogT.tile([NK, S_MACRO], f32)
                    nc.tensor.matmul(
                        lgT_ps, ones_bf,
                        maddT[:, mt * S_MACRO:(mt + 1) * S_MACRO],
                        start=True, stop=False)
                    for c in range(4):
                        nc.tensor.matmul(lgT_ps, pkT[:, c, :], enc_t[:, c, :],
                                         start=False, stop=(c == 3))
                    lgT_sb = work.tile([NK, S_MACRO], f32)
                    nc.vector.tensor_copy(out=lgT_sb, in_=lgT_ps)

                    o4_sb = outp.tile([128, NST, HID], out_dt)

                    for pr in range(NST // 2):
                        j0 = mt * NST + 2 * pr
                        # transpose two subtiles into one [128, 2, 80] psum
                        lg_ps = pslg.tile([128, 2, NK], f32)
                        for i in range(2):
                            c0 = (2 * pr + i) * S_SUB
                            nc.tensor.matmul(
                                lg_ps[:, i, :], lgT_sb[:, c0:c0 + S_SUB],
                                ident[0:NK, 0:NK], is_transpose=True,
                                start=True, stop=True)

                        # softmax pieces, pair-batched, reading psum direct
                        nmax = work.tile([128, 2], f32)
                        nc.vector.tensor_reduce(
                            out=nmax, in_=lg_ps[:, :, 0:N], axis=AX.X,
                            op=OP.max, negate=True)
                        kmax = work.tile([128, 2], f32)
                        nc.vector.tensor_reduce(
                            out=kmax, in_=lg_ps[:, :, N:NK], axis=AX.X,
                            op=OP.max, negate=True)
                        e_in = work.tile([128, 2, NK], f32)
                        nc.vector.tensor_add(
                            e_in[:, :, 0:N], lg_ps[:, :, 0:N],
                            nmax.broadcast_to([128, 2, N]))
                        nc.vector.tensor_add(
                            e_in[:, :, N:NK], lg_ps[:, :, N:NK],
                            kmax.broadcast_to([128, 2, K]))
                        e_out = work.tile([128, 2, NK], f32)
                        nc.scalar.activation(out=e_out, in_=e_in, func=AF.Exp)
                        nsum = work.tile([128, 2], f32)
                        nc.vector.tensor_reduce(
                            out=nsum, in_=e_out[:, :, 0:N], axis=AX.X,
                            op=OP.add)
                        ksum = work.tile([128, 2], f32)
                        nc.vector.tensor_reduce(
                            out=ksum, in_=e_out[:, :, N:NK], axis=AX.X,
                            op=OP.add)
                        rn = work.tile([128, 2], f32)
                        nc.vector.reciprocal(out=rn, in_=nsum)
                        nc.vector.reciprocal(out=kst_sb[:, j0:j0 + 2],
                                             in_=ksum)
                        ps_sc = work.tile([128, 2, N], f32)
                        nc.vector.tensor_mul(
                            ps_sc, e_out[:, :, 0:N],
                            rn.broadcast_to([128, 2, N]))

                        # one [128,128] transpose covers both subtiles:
                        # rows 0:64 = scoresT of subtile 2pr, 64:128 = 2pr+1
                        scT_ps = pst.tile([128, 128], f32, tag="tp")
                        nc.tensor.transpose(
                            scT_ps, ps_sc.rearrange("p a n -> p (a n)"),
                            ident)
                        scT_sb = work.tile([128, 128], bf16)
                        nc.vector.tensor_copy(out=scT_sb, in_=scT_ps)

                        for i in range(2):
                            st = 2 * pr + i
                            sl = slice(st * S_SUB, (st + 1) * S_SUB)
                            o_ps = psout.tile([128, HID], f32)
                            for c in range(4):
                                nc.tensor.matmul(o_ps, enc_t[:, c, sl],
                                                 Wh_t[:, c, :],
                                                 start=(c == 0), stop=False)
                            nc.tensor.matmul(
                                o_ps, scT_sb[i * N:(i + 1) * N, :],
                                projW_t[i * N:(i + 1) * N, :],
                                start=False, stop=True)
                            nc.scalar.activation(out=o4_sb[:, st, :],
                                                 in_=o_ps, func=AF.Tanh)

                    nc.sync.dma_start(out=out_d[b, mt], in_=o4_sb)

                kT_ps = pst.tile([16, 128], f32, tag="tp")
                nc.tensor.transpose(kT_ps, kst_sb, ident)
                kT_sb = samp.tile([16, 128], f32)
                nc.vector.tensor_copy(out=kT_sb, in_=kT_ps)
                nc.sync.dma_start(out=ks_v[b], in_=kT_sb)

    nc.compile()
    return nc


def _get_nc():
    if "nc" not in _CACHE:
        _CACHE["nc"] = _build()
    return _CACHE["nc"]


def kernel(enc_outputs, x, key_concepts, mask_enc, W_att, b_att, W_hid, b_hid,
           emb_table):
    _install_ntff_shim()
    import ml_dtypes
    from concourse.bass_utils import run_bass_kernel_spmd

    bf = ml_dtypes.bfloat16
    enc_outputs = np.asarray(enc_outputs, dtype=np.float32)
    x = np.asarray(x, dtype=np.float32)
    key_concepts = np.asarray(key_concepts)
    mask_enc = np.asarray(mask_enc)
    W_att = np.asarray(W_att, dtype=np.float32)
    b_att = np.asarray(b_att, dtype=np.float32)
    W_hid = np.asarray(W_hid, dtype=np.float32)
    b_hid = np.asarray(b_hid, dtype=np.float32)
    emb_table = np.asarray(emb_table, dtype=np.float32)

    scale = np.float32(D ** -0.5)
    # host prep: packed enc layout [b, mt, p, c, s_loc], gather, mask encoding
    # encN[b, mt, p, c, s] = enc[b, mt*512 + s, c*128 + p]
    encN = np.ascontiguousarray(
        enc_outputs.reshape(B, NMT, S_MACRO, 4, 128).transpose(0, 1, 4, 3, 2)
    ).astype(bf)
    xT = np.ascontiguousarray(x.transpose(0, 2, 1)).astype(bf)    # [B,2D,N]
    k_emb = emb_table[key_concepts.astype(np.int64)] * scale      # [B,K,D]
    kembT = np.ascontiguousarray(k_emb.transpose(0, 2, 1)).astype(bf)
    madd = np.where(mask_enc, np.float32(NEG), np.float32(0.0)).astype(bf)

    batt = np.ascontiguousarray(b_att.reshape(1, D)).astype(bf)
    bhid = np.ascontiguousarray(b_hid.reshape(1, HID))
    Wa = np.ascontiguousarray(W_att).astype(bf)
    Wh = np.ascontiguousarray(W_hid).astype(bf)

    in_maps = []
    for i in range(N_CORES):
        lo, hi = i * BPC, (i + 1) * BPC
        in_maps.append({
            "encN": np.ascontiguousarray(encN[lo:hi]),
            "xT": np.ascontiguousarray(xT[lo:hi]),
            "kembT": np.ascontiguousarray(kembT[lo:hi]),
            "madd": np.ascontiguousarray(madd[lo:hi]),
            "Wa": Wa,
            "Wh": Wh,
            "batt": batt,
            "bhid": bhid,
        })

    nc = _get_nc()
    res = run_bass_kernel_spmd(nc, in_maps, core_ids=list(range(N_CORES)),
                               trace=TRACE)
    LAST_RESULT["exec_time_ns"] = res.exec_time_ns
    LAST_RESULT["mean_exec_time_ns"] = res.mean_exec_time_ns

    # out device layout [BPC, mt, p, st, h] -> [BPC, S, HID]
    outs = []
    for i in range(N_CORES):
        o = np.asarray(res.results[i]["out"])
        if o.dtype != np.float32:
            o = o.astype(np.float32)
        o = o.reshape(BPC, NMT, 128, NST, HID).transpose(0, 1, 3, 2, 4)
        outs.append(o.reshape(BPC, S, HID))
    out = np.concatenate(outs, axis=0)
    ks = np.concatenate([res.results[i]["ks"] for i in range(N_CORES)], axis=0)
    return out, ks
